# revision 1
# baseline (speedup 1.0000x reference)
"""MoE (top-4 of 32 experts) Trainium2 kernel, data-parallel over 8 NeuronCores.

Problem shapes: x[4096,512] f32, expert_sel[32,512] f32, w1[32,512,128] f32,
w2[32,128,512] f32 -> y[4096,512] f32.

Strategy: shard the 4096 tokens 512-per-core (no collectives). Each core:
  - scores = x @ expert_sel.T in fp32 on PE (routing must match the
    reference's fp32 ranking exactly, so this stays fp32)
  - sigmoid on ACT; top-4 threshold per token via the DVE Max8 instruction;
    gate = sigmoid(score) * (score >= 4th_max), in bf16
  - gate rows are transposed to [E, T] with DVE 32x32 stream transposes,
    bounced through DRAM, and DMA-broadcast-read back as [128, T] per group
    of experts (SBUF APs cannot broadcast the partition dim; DRAM APs can)
  - dense expert MLPs in bf16: h = relu(x @ w1[e]) on ACT; hg = h * gate_e
    on DVE; y += hg @ w2[e] accumulated for all 32 experts in 4 persistent
    PSUM banks. L1 of the first experts runs before the routing chain (the
    gate-muls have LAG experts of slack).
  - all heavy DMAs are host-packed contiguous 2D transfers, enqueued on the
    SP HWDGE in need order (cumulative queue-sem thresholds then only cover
    transfers a consumer genuinely needs).
  - y written back as [D, T] tiles; host transposes and concatenates.
"""

import os
import numpy as np
import ml_dtypes

N, D, E, H, K = 4096, 512, 32, 128, 4
NCORES = 8
TPC = N // NCORES  # tokens per core = 512
P = 128
DC = D // P        # 4 contraction chunks of d
TT = TPC // P      # 4 token tiles of 128
G = 4              # experts per DMA group
NG = E // G
WREC = DC * H + D  # one expert's w1+w2 record (1024 cols)
BF16 = ml_dtypes.bfloat16

_CACHE = {}


def _split_waits_json(bir_bytes, max_waits=1):
    """The walrus build in this container encodes at most one sync-wait per
    instruction; Tile emits several on some. Split excess waits onto
    preceding same-engine NoOps (identical semantics: program order on the
    engine)."""
    import orjson

    bir = orjson.loads(bir_bytes)
    nid = [0]

    def fix_block(instructions):
        out = []
        for ins in instructions:
            si = ins.get("sync_info")
            waits = (si or {}).get("on_wait") or []
            if len(waits) > max_waits:
                chunks = [
                    waits[i : i + max_waits] for i in range(0, len(waits), max_waits)
                ]
                for ch in chunks[:-1]:
                    nid[0] += 1
                    out.append(
                        {
                            "opcode": "NoOp",
                            "engine": ins["engine"],
                            "ins": [],
                            "outs": [],
                            "name": f"I-ws-{nid[0]}",
                            "debug": ins.get("debug", 0),
                            "sync_info": {"on_wait": ch, "on_update": []},
                        }
                    )
                si["on_wait"] = chunks[-1]
            out.append(ins)
        return out

    def walk(o):
        if isinstance(o, dict):
            for k, v in o.items():
                if k == "instructions" and isinstance(v, list):
                    o[k] = fix_block(v)
                else:
                    walk(v)
        elif isinstance(o, list):
            for v in o:
                walk(v)

    walk(bir)
    return orjson.dumps(bir)


def _patch_compile():
    if _CACHE.get("patched"):
        return
    import concourse.bass2jax as bass2jax
    from concourse.bass_utils import compile_bir_kernel as _orig

    def patched(bir_json, tmpdir, neff_name="file.neff"):
        return _orig(_split_waits_json(bir_json), tmpdir, neff_name=neff_name)

    bass2jax.compile_bir_kernel = patched
    _CACHE["patched"] = True


def _patch_tile_tail():
    # Tile's kernel epilogue is drain + barrier + sem-clears + barrier. The
    # second barrier only orders the clears vs engine program-end; NRT
    # already requires every engine's program to finish before the NEFF can
    # run again, so it is dead time (~2-3us). Drop it.
    if _CACHE.get("tail_patched"):
        return
    from concourse.tile import TileContext
    from concourse.vector_clock import ScopedClock

    def _dab(self, tick_clock, wait_clock):
        drain_inst = self.nc.sync.drain()
        wait_clock.add_sem_waits(
            drain_inst.ins, ScopedClock({None: tick_clock.global_clock})
        )
        self.nc.all_engine_barrier()
        popped = self.nc._tile_sem_poison_stack.pop()
        assert popped is self._sem_poison
        self.nc.clear_and_free_semaphores(list(self.sems.allocated().values()))

    TileContext._drain_and_barrier = _dab
    _CACHE["tail_patched"] = True


def _build_nc():
    import concourse.bass as bass
    import concourse.mybir as mybir
    from concourse.tile import TileContext

    _patch_tile_tail()

    dt = mybir.dt
    Alu = mybir.AluOpType
    Act = mybir.ActivationFunctionType

    nc = bass.Bass()

    xT_d = nc.dram_tensor("xT", [P, DC * TPC], dt.float32, kind="ExternalInput")
    xbT_d = nc.dram_tensor("xbT", [P, DC * TPC], dt.bfloat16, kind="ExternalInput")
    selT_d = nc.dram_tensor("selT", [P, DC * E], dt.float32, kind="ExternalInput")
    wc_d = nc.dram_tensor("wc", [NG, P, G * WREC], dt.bfloat16, kind="ExternalInput")
    yT_d = nc.dram_tensor("yT", [DC, P, TPC], dt.bfloat16, kind="ExternalOutput")

    with TileContext(nc) as tc:
        with (
            tc.tile_pool(name="singles", bufs=1) as singles,
            tc.tile_pool(name="dram", bufs=1, space="DRAM") as drampool,
            tc.tile_pool(name="wpool", bufs=8) as wpool,
            tc.tile_pool(name="gpool", bufs=8) as gpool,
            tc.tile_pool(name="hrpool", bufs=16) as hrpool,
            tc.tile_pool(name="hgpool", bufs=16) as hgpool,
            tc.tile_pool(name="ph", bufs=3, space="PSUM") as phpool,
            tc.tile_pool(name="py", bufs=1, space="PSUM") as pypool,
        ):
            TH = TPC // 2
            xfA = singles.tile([P, DC * TH], dt.float32)
            xfB = singles.tile([P, DC * TH], dt.float32)
            xb = singles.tile([P, DC * TPC], dt.bfloat16)
            sel = singles.tile([P, DC * E], dt.float32)
            ssb = singles.tile([P, TT * E], dt.float32)
            sig = singles.tile([P, TT * E], dt.float32)
            m8 = singles.tile([P, TT * 8], dt.float32)
            gate = singles.tile([P, TT * E], dt.bfloat16)
            gTb = singles.tile([32, TPC], dt.bfloat16)
            y_sb = singles.tile([P, DC * TPC], dt.bfloat16)
            gTd = drampool.tile([E, TPC], dt.bfloat16)

            py = pypool.tile([P, DC, TPC], dt.float32)

            wgrp = {}
            ggrp = {}
            hgs = {}
            hrs = {}

            # PE warm-up: junk matmuls on uninitialized SBUF while the input
            # DMAs stream in, so the HAM clock gate is at 8/8 (2.4 GHz) when
            # the real matmul stream starts. Results are discarded.
            junk = singles.tile([P, TPC], dt.bfloat16)
            nc.gpsimd.memset(junk[:], 1.0)
            pj = phpool.tile([P, TPC], dt.float32, tag="ph", name="pjunk")
            for _ in range(24):
                nc.tensor.matmul(pj[:], junk[:, :P], junk[:], start=True, stop=True)

            def dma_group(gi):
                wt = wpool.tile([P, G * WREC], dt.bfloat16, tag="wt", name=f"wt{gi}")
                nc.sync.dma_start(wt[:], wc_d[gi])
                wgrp[gi] = wt

            def g_group(gi):
                g = gpool.tile([P, G * TPC], dt.bfloat16, tag="g", name=f"g{gi}")
                base = gTd[gi * G : (gi + 1) * G, :]
                src = bass.AP(base.tensor, base.offset, [[0, P]] + list(base.ap))
                nc.sync.dma_start(g[:].rearrange("p (e t) -> p e t", e=G), src)
                ggrp[gi] = g

            def scores_section():
                for tt in range(TT):
                    psc = phpool.tile([P, E], dt.float32, tag="ph", name=f"psc{tt}")
                    xh = xfA if tt < 2 else xfB
                    tl = (tt % 2) * P
                    for dc in range(DC):
                        nc.tensor.matmul(
                            psc[:],
                            xh[:, dc * TH + tl : dc * TH + tl + P],
                            sel[:, dc * E : (dc + 1) * E],
                            start=(dc == 0),
                            stop=(dc == DC - 1),
                        )
                    sl = slice(tt * E, (tt + 1) * E)
                    nc.scalar.activation(sig[:, sl], psc[:], Act.Sigmoid)
                    nc.vector.tensor_copy(ssb[:, sl], psc[:])
                    nc.vector.max(m8[:, tt * 8 : (tt + 1) * 8], ssb[:, sl])
                    # gate = (score >= 4th max) * sigmoid(score), in bf16
                    nc.vector.scalar_tensor_tensor(
                        gate[:, sl],
                        ssb[:, sl],
                        m8[:, tt * 8 + 3 : tt * 8 + 4],
                        sig[:, sl],
                        op0=Alu.is_ge,
                        op1=Alu.mult,
                    )
                    # transpose this token-tile of the gate to [E, T]
                    for pb in range(TT):
                        nc.vector.transpose(
                            gTb[0:32, tt * P + pb * 32 : tt * P + (pb + 1) * 32],
                            gate[pb * 32 : (pb + 1) * 32, sl],
                        )
                nc.gpsimd.dma_start(gTd[:], gTb[0:32, :])

            def l1_mm(e):
                gi, ei = divmod(e, G)
                wt = wgrp[gi]
                ph = phpool.tile([P, TPC], dt.float32, tag="ph", name=f"ph{e}")
                for dc in range(DC):
                    nc.tensor.matmul(
                        ph[:],
                        wt[:, ei * WREC + dc * H : ei * WREC + (dc + 1) * H],
                        xb[:, dc * TPC : (dc + 1) * TPC],
                        start=(dc == 0),
                        stop=(dc == DC - 1),
                    )
                hr = hrpool.tile([P, TPC], dt.bfloat16, tag="hr", name=f"hr{e}")
                nc.scalar.activation(hr[:], ph[:], Act.Relu)
                hrs[e] = hr

            def l1_mul(e):
                gi, ei = divmod(e, G)
                hr = hrs.pop(e)
                hg = hgpool.tile([P, TPC], dt.bfloat16, tag="hg", name=f"hg{e}")
                nc.vector.tensor_mul(
                    hg[:], hr[:], ggrp[gi][:, ei * TPC : (ei + 1) * TPC]
                )
                hgs[e] = hg

            def l2_part(e):
                gi, ei = divmod(e, G)
                hg = hgs.pop(e)
                wt = wgrp[gi]
                base = ei * WREC + DC * H
                for dtile in range(DC):
                    nc.tensor.matmul(
                        py[:, dtile, :],
                        wt[:, base + dtile * P : base + (dtile + 1) * P],
                        hg[:],
                        start=(e == 0),
                        stop=(e == E - 1),
                        skip_group_check=True,
                    )

            LAG = 14
            # SP HWDGE enqueue in need order; every weight group precedes any
            # gate read so weight waits never include gate transfers. The
            # routing inputs go first: the gate path is the long pole.
            nc.sync.dma_start(sel[:], selT_d[:])
            src = xT_d[:].rearrange("p (c t) -> p c t", c=DC)
            nc.sync.dma_start(
                xfA[:].rearrange("p (c t) -> p c t", c=DC), src[:, :, :TH]
            )
            nc.sync.dma_start(
                xfB[:].rearrange("p (c t) -> p c t", c=DC), src[:, :, TH:]
            )
            nc.sync.dma_start(xb[:], xbT_d[:])
            for gi in range(NG):
                dma_group(gi)
            scores_section()
            for gi in range(NG):
                g_group(gi)
            for e in range(E + LAG):
                if e < E:
                    l1_mm(e)
                    l1_mul(e)
                if e >= LAG:
                    l2_part(e - LAG)

            # --- evict y and store (copies split across DVE/ACT) ---
            for dtile in range(DC):
                sl = slice(dtile * TPC, (dtile + 1) * TPC)
                if dtile % 2 == 0:
                    nc.vector.tensor_copy(y_sb[:, sl], py[:, dtile, :])
                else:
                    nc.scalar.activation(y_sb[:, sl], py[:, dtile, :], Act.Copy)
                nc.sync.dma_start(yT_d[dtile], y_sb[:, sl])

    return nc


def _get_nc():
    if "nc" not in _CACHE:
        _CACHE["nc"] = _build_nc()
    return _CACHE["nc"]


def _pack_inputs(x, expert_sel, w1, w2):
    x = np.asarray(x, dtype=np.float32)
    expert_sel = np.asarray(expert_sel, dtype=np.float32)
    w1 = np.asarray(w1, dtype=np.float32)
    w2 = np.asarray(w2, dtype=np.float32)

    # selT: [p, dc*E + e] = expert_sel[e, dc*P + p]
    selT = np.ascontiguousarray(
        expert_sel.T.reshape(DC, P, E).transpose(1, 0, 2)
    ).reshape(P, DC * E)
    # per-expert record [p, dc*H + h | DC*H + d], grouped by G experts
    w1p = (
        w1.astype(BF16).reshape(E, DC, P, H).transpose(0, 2, 1, 3).reshape(E, P, DC * H)
    )
    w2p = w2.astype(BF16)
    wc = np.concatenate([w1p, w2p], axis=2)  # [E, P, WREC]
    wc = np.ascontiguousarray(
        wc.reshape(NG, G, P, WREC).transpose(0, 2, 1, 3)
    ).reshape(NG, P, G * WREC)

    in_maps = []
    for c in range(NCORES):
        xc = x[c * TPC : (c + 1) * TPC]
        # xT: [p, dc*TPC + t] = x[t, dc*P + p]
        xT = np.ascontiguousarray(
            xc.T.reshape(DC, P, TPC).transpose(1, 0, 2)
        ).reshape(P, DC * TPC)
        in_maps.append({"xT": xT, "xbT": xT.astype(BF16), "selT": selT, "wc": wc})
    return in_maps


def _run(x, expert_sel, w1, w2, trace=False, tmpdir=None):
    _patch_compile()
    from concourse.bass_utils import run_bass_kernel_spmd

    if trace:
        _install_ntff_hook()

    nc = _get_nc()
    in_maps = _pack_inputs(x, expert_sel, w1, w2)
    res = run_bass_kernel_spmd(
        nc, in_maps, list(range(NCORES)), trace=trace, tmpdir=tmpdir
    )
    y = np.empty((N, D), dtype=np.float32)
    for c in range(NCORES):
        yT = np.asarray(res.results[c]["yT"], dtype=np.float32)
        y[c * TPC : (c + 1) * TPC] = yT.reshape(D, TPC).T
    return y, res


def _install_ntff_hook():
    """Register the NTFF profiling hook (the container's antenv stub lacks
    axon_hooks; replicate trn_boot's ctypes hook). Also stub the artifact
    upload, which needs cloud storage not present here."""
    if _CACHE.get("ntff"):
        return
    import sys, types, ctypes, contextlib
    import antenv  # noqa: F401
    import concourse.bass_utils as bass_utils

    bass_utils.upload_artifacts = lambda d: f"file://{d}"

    mod = types.ModuleType("antenv.axon_hooks")
    store = [None]
    mod.set_axon_ntff_profile_hook = lambda h: store.__setitem__(0, h)
    mod.get_axon_ntff_profile_hook = lambda: store[0]
    sys.modules["antenv.axon_hooks"] = mod

    lib = ctypes.CDLL("/opt/axon/libaxon_pjrt.so")
    if hasattr(lib, "axon_start_nrt_profile"):
        lib.axon_start_nrt_profile.argtypes = [
            ctypes.POINTER(ctypes.c_int64),
            ctypes.c_size_t,
        ]
        lib.axon_start_nrt_profile.restype = ctypes.c_int64
        lib.axon_stop_nrt_profile.argtypes = [ctypes.c_char_p]
        lib.axon_stop_nrt_profile.restype = ctypes.c_int64

        @contextlib.contextmanager
        def _hook(output_dir, device_ids):
            import jax

            jax.devices()
            if device_ids:
                ids = (ctypes.c_int64 * len(device_ids))(*device_ids)
                rc = lib.axon_start_nrt_profile(ids, len(device_ids))
            else:
                rc = lib.axon_start_nrt_profile(None, 0)
            if rc != 0:
                raise RuntimeError(f"axon_start_nrt_profile rc={rc}")
            try:
                yield
            finally:
                n = lib.axon_stop_nrt_profile(str(output_dir).encode())
                if n <= 0:
                    print(f"ntff profile wrote {n} files", flush=True)

        mod.set_axon_ntff_profile_hook(_hook)
    _CACHE["ntff"] = True


def kernel(x, expert_sel, w1, w2):
    y, _ = _run(x, expert_sel, w1, w2, trace=False)
    return y



# revision 3
# speedup vs baseline: 1.2628x; 1.2628x over previous
"""MoE (top-4 of 32 experts) Trainium2 kernel, data-parallel over 8 NeuronCores.

Problem shapes: x[4096,512] f32, expert_sel[32,512] f32, w1[32,512,128] f32,
w2[32,128,512] f32 -> y[4096,512] f32.

Strategy: shard the 4096 tokens 512-per-core (no collectives). Each core:
  - scores = x @ expert_sel.T in fp32 on PE (routing must match the
    reference's fp32 ranking exactly, so this stays fp32)
  - sigmoid on ACT; top-4 threshold per token via the DVE Max8 instruction;
    gate = sigmoid(score) * (score >= 4th_max), in bf16
  - gate rows are transposed to [E, T] with DVE 32x32 stream transposes,
    bounced through DRAM, and DMA-broadcast-read back as [128, T] per group
    of experts (SBUF APs cannot broadcast the partition dim; DRAM APs can)
  - dense expert MLPs in bf16: h = relu(x @ w1[e]) on ACT; hg = h * gate_e
    on DVE; y += hg @ w2[e] accumulated for all 32 experts in 4 persistent
    PSUM banks. L1 of the first experts runs before the routing chain (the
    gate-muls have LAG experts of slack).
  - all heavy DMAs are host-packed contiguous 2D transfers, enqueued on the
    SP HWDGE in need order (cumulative queue-sem thresholds then only cover
    transfers a consumer genuinely needs).
  - y written back as [D, T] tiles; host transposes and concatenates.
"""

import os
import numpy as np
import ml_dtypes

N, D, E, H, K = 4096, 512, 32, 128, 4
NCORES = 8
TPC = N // NCORES  # tokens per core = 512
P = 128
DC = D // P        # 4 contraction chunks of d
TT = TPC // P      # 4 token tiles of 128
G = 4              # experts per DMA group
NG = E // G
WREC = DC * H + D  # one expert's w1+w2 record (1024 cols)
BF16 = ml_dtypes.bfloat16

_CACHE = {}


def _split_waits_json(bir_bytes, max_waits=1):
    """The walrus build in this container encodes at most one sync-wait per
    instruction; Tile emits several on some. Split excess waits onto
    preceding same-engine NoOps (identical semantics: program order on the
    engine)."""
    import orjson

    bir = orjson.loads(bir_bytes)
    nid = [0]

    def fix_block(instructions):
        out = []
        for ins in instructions:
            si = ins.get("sync_info")
            waits = (si or {}).get("on_wait") or []
            if len(waits) > max_waits:
                chunks = [
                    waits[i : i + max_waits] for i in range(0, len(waits), max_waits)
                ]
                for ch in chunks[:-1]:
                    nid[0] += 1
                    out.append(
                        {
                            "opcode": "NoOp",
                            "engine": ins["engine"],
                            "ins": [],
                            "outs": [],
                            "name": f"I-ws-{nid[0]}",
                            "debug": ins.get("debug", 0),
                            "sync_info": {"on_wait": ch, "on_update": []},
                        }
                    )
                si["on_wait"] = chunks[-1]
            out.append(ins)
        return out

    def walk(o):
        if isinstance(o, dict):
            for k, v in o.items():
                if k == "instructions" and isinstance(v, list):
                    o[k] = fix_block(v)
                else:
                    walk(v)
        elif isinstance(o, list):
            for v in o:
                walk(v)

    walk(bir)
    return orjson.dumps(bir)


def _patch_compile():
    if _CACHE.get("patched"):
        return
    import concourse.bass2jax as bass2jax
    from concourse.bass_utils import compile_bir_kernel as _orig

    def patched(bir_json, tmpdir, neff_name="file.neff"):
        return _orig(_split_waits_json(bir_json), tmpdir, neff_name=neff_name)

    bass2jax.compile_bir_kernel = patched
    _CACHE["patched"] = True


def _patch_tile_tail():
    # Tile's kernel epilogue is drain + barrier + sem-clears + barrier. The
    # second barrier only orders the clears vs engine program-end; NRT
    # already requires every engine's program to finish before the NEFF can
    # run again, so it is dead time (~2-3us). Drop it.
    if _CACHE.get("tail_patched"):
        return
    from concourse.tile import TileContext
    from concourse.vector_clock import ScopedClock

    def _dab(self, tick_clock, wait_clock):
        drain_inst = self.nc.sync.drain()
        wait_clock.add_sem_waits(
            drain_inst.ins, ScopedClock({None: tick_clock.global_clock})
        )
        self.nc.all_engine_barrier()
        popped = self.nc._tile_sem_poison_stack.pop()
        assert popped is self._sem_poison
        self.nc.clear_and_free_semaphores(list(self.sems.allocated().values()))

    TileContext._drain_and_barrier = _dab
    _CACHE["tail_patched"] = True


def _build_nc():
    import concourse.bass as bass
    import concourse.mybir as mybir
    from concourse.tile import TileContext

    _patch_tile_tail()

    dt = mybir.dt
    Alu = mybir.AluOpType
    Act = mybir.ActivationFunctionType

    nc = bass.Bass()

    junk_d = nc.dram_tensor("junkc", [P, TPC], dt.bfloat16, kind="ExternalInput")
    xT_d = nc.dram_tensor("xT", [P, DC * TPC], dt.float32, kind="ExternalInput")
    xbT_d = nc.dram_tensor("xbT", [P, DC * TPC], dt.bfloat16, kind="ExternalInput")
    selT_d = nc.dram_tensor("selT", [P, DC * E], dt.float32, kind="ExternalInput")
    wc_d = nc.dram_tensor("wc", [NG, P, G * WREC], dt.bfloat16, kind="ExternalInput")
    yT_d = nc.dram_tensor("yT", [DC, P, TPC], dt.bfloat16, kind="ExternalOutput")

    with TileContext(nc) as tc:
        with (
            tc.tile_pool(name="singles", bufs=1) as singles,
            tc.tile_pool(name="dram", bufs=1, space="DRAM") as drampool,
            tc.tile_pool(name="wpool", bufs=8) as wpool,
            tc.tile_pool(name="gpool", bufs=8) as gpool,
            tc.tile_pool(name="hrpool", bufs=16) as hrpool,
            tc.tile_pool(name="hgpool", bufs=16) as hgpool,
            tc.tile_pool(name="ph", bufs=3, space="PSUM") as phpool,
            tc.tile_pool(name="py", bufs=1, space="PSUM") as pypool,
        ):
            TH = TPC // 2
            xfA = singles.tile([P, DC * TH], dt.float32)
            xfB = singles.tile([P, DC * TH], dt.float32)
            xb = singles.tile([P, DC * TPC], dt.bfloat16)
            sel = singles.tile([P, DC * E], dt.float32)
            ssb = singles.tile([P, TT * E], dt.float32)
            sig = singles.tile([P, TT * E], dt.float32)
            m8 = singles.tile([P, TT * 8], dt.float32)
            gate = singles.tile([P, TT * E], dt.bfloat16)
            gTb = singles.tile([32, TPC], dt.bfloat16)
            y_sb = singles.tile([P, DC * TPC], dt.bfloat16)
            gTd = drampool.tile([E, TPC], dt.bfloat16)

            py = pypool.tile([P, DC, TPC], dt.float32)

            wgrp = {}
            ggrp = {}
            hgs = {}
            hrs = {}

            # PE warm-up: junk matmuls on uninitialized SBUF while the input
            # DMAs stream in, so the HAM clock gate is at 8/8 (2.4 GHz) when
            # the real matmul stream starts. Results are discarded.
            junk = singles.tile([P, TPC], dt.bfloat16)
            nc.sync.dma_start(junk[:], junk_d[:])
            pj = phpool.tile([P, TPC], dt.float32, tag="ph", name="pjunk")
            for _ in range(24):
                nc.tensor.matmul(pj[:], junk[:, :P], junk[:], start=True, stop=True)

            def dma_group(gi):
                wt = wpool.tile([P, G * WREC], dt.bfloat16, tag="wt", name=f"wt{gi}")
                nc.sync.dma_start(wt[:], wc_d[gi])
                wgrp[gi] = wt

            def g_group(gi):
                g = gpool.tile([P, G * TPC], dt.bfloat16, tag="g", name=f"g{gi}")
                base = gTd[gi * G : (gi + 1) * G, :]
                src = bass.AP(base.tensor, base.offset, [[0, P]] + list(base.ap))
                nc.sync.dma_start(g[:].rearrange("p (e t) -> p e t", e=G), src)
                ggrp[gi] = g

            def scores_section():
                for tt in range(TT):
                    psc = phpool.tile([P, E], dt.float32, tag="ph", name=f"psc{tt}")
                    xh = xfA if tt < 2 else xfB
                    tl = (tt % 2) * P
                    for dc in range(DC):
                        nc.tensor.matmul(
                            psc[:],
                            xh[:, dc * TH + tl : dc * TH + tl + P],
                            sel[:, dc * E : (dc + 1) * E],
                            start=(dc == 0),
                            stop=(dc == DC - 1),
                        )
                    sl = slice(tt * E, (tt + 1) * E)
                    nc.scalar.activation(sig[:, sl], psc[:], Act.Sigmoid)
                    nc.vector.tensor_copy(ssb[:, sl], psc[:])
                    nc.vector.max(m8[:, tt * 8 : (tt + 1) * 8], ssb[:, sl])
                    # gate = (score >= 4th max) * sigmoid(score), in bf16
                    nc.vector.scalar_tensor_tensor(
                        gate[:, sl],
                        ssb[:, sl],
                        m8[:, tt * 8 + 3 : tt * 8 + 4],
                        sig[:, sl],
                        op0=Alu.is_ge,
                        op1=Alu.mult,
                    )
                    # transpose this token-tile of the gate to [E, T]
                    for pb in range(TT):
                        nc.vector.transpose(
                            gTb[0:32, tt * P + pb * 32 : tt * P + (pb + 1) * 32],
                            gate[pb * 32 : (pb + 1) * 32, sl],
                        )
                nc.gpsimd.dma_start(gTd[:], gTb[0:32, :])

            def l1_mm(e):
                gi, ei = divmod(e, G)
                wt = wgrp[gi]
                ph = phpool.tile([P, TPC], dt.float32, tag="ph", name=f"ph{e}")
                for dc in range(DC):
                    nc.tensor.matmul(
                        ph[:],
                        wt[:, ei * WREC + dc * H : ei * WREC + (dc + 1) * H],
                        xb[:, dc * TPC : (dc + 1) * TPC],
                        start=(dc == 0),
                        stop=(dc == DC - 1),
                    )
                hr = hrpool.tile([P, TPC], dt.bfloat16, tag="hr", name=f"hr{e}")
                nc.scalar.activation(hr[:], ph[:], Act.Relu)
                hrs[e] = hr

            def l1_mul(e):
                gi, ei = divmod(e, G)
                hr = hrs.pop(e)
                hg = hgpool.tile([P, TPC], dt.bfloat16, tag="hg", name=f"hg{e}")
                nc.vector.tensor_mul(
                    hg[:], hr[:], ggrp[gi][:, ei * TPC : (ei + 1) * TPC]
                )
                hgs[e] = hg

            def l2_part(e):
                gi, ei = divmod(e, G)
                hg = hgs.pop(e)
                wt = wgrp[gi]
                base = ei * WREC + DC * H
                for dtile in range(DC):
                    nc.tensor.matmul(
                        py[:, dtile, :],
                        wt[:, base + dtile * P : base + (dtile + 1) * P],
                        hg[:],
                        start=(e == 0),
                        stop=(e == E - 1),
                        skip_group_check=True,
                    )

            LAG = 14
            # SP HWDGE enqueue in need order; every weight group precedes any
            # gate read so weight waits never include gate transfers. The
            # routing inputs go first: the gate path is the long pole.
            nc.sync.dma_start(sel[:], selT_d[:])
            src = xT_d[:].rearrange("p (c t) -> p c t", c=DC)
            nc.sync.dma_start(
                xfA[:].rearrange("p (c t) -> p c t", c=DC), src[:, :, :TH]
            )
            nc.sync.dma_start(
                xfB[:].rearrange("p (c t) -> p c t", c=DC), src[:, :, TH:]
            )
            nc.sync.dma_start(xb[:], xbT_d[:])
            for gi in range(NG):
                dma_group(gi)
            scores_section()
            for gi in range(NG):
                g_group(gi)
            for e in range(E + LAG):
                if e < E:
                    l1_mm(e)
                    l1_mul(e)
                if e >= LAG:
                    l2_part(e - LAG)

            # --- evict y and store (copies split across DVE/ACT) ---
            for dtile in range(DC):
                sl = slice(dtile * TPC, (dtile + 1) * TPC)
                if dtile % 2 == 0:
                    nc.vector.tensor_copy(y_sb[:, sl], py[:, dtile, :])
                else:
                    nc.scalar.activation(y_sb[:, sl], py[:, dtile, :], Act.Copy)
                nc.sync.dma_start(yT_d[dtile], y_sb[:, sl])

    return nc


def _get_nc():
    if "nc" not in _CACHE:
        _CACHE["nc"] = _build_nc()
    return _CACHE["nc"]


def _pack_inputs(x, expert_sel, w1, w2):
    x = np.asarray(x, dtype=np.float32)
    expert_sel = np.asarray(expert_sel, dtype=np.float32)
    w1 = np.asarray(w1, dtype=np.float32)
    w2 = np.asarray(w2, dtype=np.float32)

    # selT: [p, dc*E + e] = expert_sel[e, dc*P + p]
    selT = np.ascontiguousarray(
        expert_sel.T.reshape(DC, P, E).transpose(1, 0, 2)
    ).reshape(P, DC * E)
    # per-expert record [p, dc*H + h | DC*H + d], grouped by G experts
    w1p = (
        w1.astype(BF16).reshape(E, DC, P, H).transpose(0, 2, 1, 3).reshape(E, P, DC * H)
    )
    w2p = w2.astype(BF16)
    wc = np.concatenate([w1p, w2p], axis=2)  # [E, P, WREC]
    wc = np.ascontiguousarray(
        wc.reshape(NG, G, P, WREC).transpose(0, 2, 1, 3)
    ).reshape(NG, P, G * WREC)

    junkc = np.ones((P, TPC), dtype=BF16)
    in_maps = []
    for c in range(NCORES):
        xc = x[c * TPC : (c + 1) * TPC]
        # xT: [p, dc*TPC + t] = x[t, dc*P + p]
        xT = np.ascontiguousarray(
            xc.T.reshape(DC, P, TPC).transpose(1, 0, 2)
        ).reshape(P, DC * TPC)
        in_maps.append({"xT": xT, "xbT": xT.astype(BF16), "selT": selT, "wc": wc,
                        "junkc": junkc})
    return in_maps


def _run(x, expert_sel, w1, w2, trace=False, tmpdir=None):
    _patch_compile()
    from concourse.bass_utils import run_bass_kernel_spmd

    if trace:
        _install_ntff_hook()

    nc = _get_nc()
    in_maps = _pack_inputs(x, expert_sel, w1, w2)
    res = run_bass_kernel_spmd(
        nc, in_maps, list(range(NCORES)), trace=trace, tmpdir=tmpdir
    )
    y = np.empty((N, D), dtype=np.float32)
    for c in range(NCORES):
        yT = np.asarray(res.results[c]["yT"], dtype=np.float32)
        y[c * TPC : (c + 1) * TPC] = yT.reshape(D, TPC).T
    return y, res


def _install_ntff_hook():
    """Register the NTFF profiling hook (the container's antenv stub lacks
    axon_hooks; replicate trn_boot's ctypes hook). Also stub the artifact
    upload, which needs cloud storage not present here."""
    if _CACHE.get("ntff"):
        return
    import sys, types, ctypes, contextlib
    import antenv  # noqa: F401
    import concourse.bass_utils as bass_utils

    bass_utils.upload_artifacts = lambda d: f"file://{d}"

    mod = types.ModuleType("antenv.axon_hooks")
    store = [None]
    mod.set_axon_ntff_profile_hook = lambda h: store.__setitem__(0, h)
    mod.get_axon_ntff_profile_hook = lambda: store[0]
    sys.modules["antenv.axon_hooks"] = mod

    lib = ctypes.CDLL("/opt/axon/libaxon_pjrt.so")
    if hasattr(lib, "axon_start_nrt_profile"):
        lib.axon_start_nrt_profile.argtypes = [
            ctypes.POINTER(ctypes.c_int64),
            ctypes.c_size_t,
        ]
        lib.axon_start_nrt_profile.restype = ctypes.c_int64
        lib.axon_stop_nrt_profile.argtypes = [ctypes.c_char_p]
        lib.axon_stop_nrt_profile.restype = ctypes.c_int64

        @contextlib.contextmanager
        def _hook(output_dir, device_ids):
            import jax

            jax.devices()
            if device_ids:
                ids = (ctypes.c_int64 * len(device_ids))(*device_ids)
                rc = lib.axon_start_nrt_profile(ids, len(device_ids))
            else:
                rc = lib.axon_start_nrt_profile(None, 0)
            if rc != 0:
                raise RuntimeError(f"axon_start_nrt_profile rc={rc}")
            try:
                yield
            finally:
                n = lib.axon_stop_nrt_profile(str(output_dir).encode())
                if n <= 0:
                    print(f"ntff profile wrote {n} files", flush=True)

        mod.set_axon_ntff_profile_hook(_hook)
    _CACHE["ntff"] = True


def kernel(x, expert_sel, w1, w2):
    y, _ = _run(x, expert_sel, w1, w2, trace=False)
    return y



# revision 4
# speedup vs baseline: 1.4026x; 1.1107x over previous
"""MoE (top-4 of 32 experts) Trainium2 kernel, data-parallel over 8 NeuronCores.

Problem shapes: x[4096,512] f32, expert_sel[32,512] f32, w1[32,512,128] f32,
w2[32,128,512] f32 -> y[4096,512] f32.

Strategy: shard the 4096 tokens 512-per-core (no collectives). Each core:
  - scores = x @ expert_sel.T in fp32 on PE (routing must match the
    reference's fp32 ranking exactly, so this stays fp32)
  - sigmoid on ACT; top-4 threshold per token via the DVE Max8 instruction;
    gate = sigmoid(score) * (score >= 4th_max), in bf16
  - gate rows are transposed to [E, T] with DVE 32x32 stream transposes,
    bounced through DRAM, and DMA-broadcast-read back as [128, T] per group
    of experts (SBUF APs cannot broadcast the partition dim; DRAM APs can)
  - dense expert MLPs in bf16: h = relu(x @ w1[e]) on ACT; hg = h * gate_e
    on DVE; y += hg @ w2[e] accumulated for all 32 experts in 4 persistent
    PSUM banks. L1 of the first experts runs before the routing chain (the
    gate-muls have LAG experts of slack).
  - all heavy DMAs are host-packed contiguous 2D transfers, enqueued on the
    SP HWDGE in need order (cumulative queue-sem thresholds then only cover
    transfers a consumer genuinely needs).
  - y written back as [D, T] tiles; host transposes and concatenates.
"""

import os
import numpy as np
import ml_dtypes

N, D, E, H, K = 4096, 512, 32, 128, 4
NCORES = 8
TPC = N // NCORES  # tokens per core = 512
P = 128
DC = D // P        # 4 contraction chunks of d
TT = TPC // P      # 4 token tiles of 128
G = 4              # experts per DMA group
NG = E // G
WREC = DC * H + D  # one expert's w1+w2 record (1024 cols)
BF16 = ml_dtypes.bfloat16

_CACHE = {}


def _split_waits_json(bir_bytes, max_waits=1):
    """The walrus build in this container encodes at most one sync-wait per
    instruction; Tile emits several on some. Split excess waits onto
    preceding same-engine NoOps (identical semantics: program order on the
    engine)."""
    import orjson

    bir = orjson.loads(bir_bytes)
    nid = [0]

    def fix_block(instructions):
        out = []
        for ins in instructions:
            si = ins.get("sync_info")
            waits = (si or {}).get("on_wait") or []
            if len(waits) > max_waits:
                chunks = [
                    waits[i : i + max_waits] for i in range(0, len(waits), max_waits)
                ]
                for ch in chunks[:-1]:
                    nid[0] += 1
                    out.append(
                        {
                            "opcode": "NoOp",
                            "engine": ins["engine"],
                            "ins": [],
                            "outs": [],
                            "name": f"I-ws-{nid[0]}",
                            "debug": ins.get("debug", 0),
                            "sync_info": {"on_wait": ch, "on_update": []},
                        }
                    )
                si["on_wait"] = chunks[-1]
            out.append(ins)
        return out

    def walk(o):
        if isinstance(o, dict):
            for k, v in o.items():
                if k == "instructions" and isinstance(v, list):
                    o[k] = fix_block(v)
                else:
                    walk(v)
        elif isinstance(o, list):
            for v in o:
                walk(v)

    walk(bir)
    return orjson.dumps(bir)


def _patch_compile():
    if _CACHE.get("patched"):
        return
    import concourse.bass2jax as bass2jax
    from concourse.bass_utils import compile_bir_kernel as _orig

    def patched(bir_json, tmpdir, neff_name="file.neff"):
        return _orig(_split_waits_json(bir_json), tmpdir, neff_name=neff_name)

    bass2jax.compile_bir_kernel = patched
    _CACHE["patched"] = True


def _patch_tile_tail():
    # Tile's kernel epilogue is drain + barrier + sem-clears + barrier. The
    # second barrier only orders the clears vs engine program-end; NRT
    # already requires every engine's program to finish before the NEFF can
    # run again, so it is dead time (~2-3us). Drop it.
    if _CACHE.get("tail_patched"):
        return
    from concourse.tile import TileContext
    from concourse.vector_clock import ScopedClock

    def _dab(self, tick_clock, wait_clock):
        drain_inst = self.nc.sync.drain()
        wait_clock.add_sem_waits(
            drain_inst.ins, ScopedClock({None: tick_clock.global_clock})
        )
        self.nc.all_engine_barrier()
        popped = self.nc._tile_sem_poison_stack.pop()
        assert popped is self._sem_poison
        self.nc.clear_and_free_semaphores(list(self.sems.allocated().values()))

    TileContext._drain_and_barrier = _dab
    _CACHE["tail_patched"] = True


def _build_nc():
    import concourse.bass as bass
    import concourse.mybir as mybir
    from concourse.tile import TileContext

    _patch_tile_tail()

    dt = mybir.dt
    Alu = mybir.AluOpType
    Act = mybir.ActivationFunctionType

    nc = bass.Bass()

    xT_d = nc.dram_tensor("xT", [P, DC * TPC], dt.float32, kind="ExternalInput")
    xbT_d = nc.dram_tensor("xbT", [P, DC * TPC], dt.bfloat16, kind="ExternalInput")
    selT_d = nc.dram_tensor("selT", [P, DC * E], dt.float32, kind="ExternalInput")
    wc_d = nc.dram_tensor("wc", [NG, P, G * WREC], dt.bfloat16, kind="ExternalInput")
    yT_d = nc.dram_tensor("yT", [DC, P, TPC], dt.bfloat16, kind="ExternalOutput")

    with TileContext(nc) as tc:
        with (
            tc.tile_pool(name="singles", bufs=1) as singles,
            tc.tile_pool(name="dram", bufs=1, space="DRAM") as drampool,
            tc.tile_pool(name="wpool", bufs=8) as wpool,
            tc.tile_pool(name="gpool", bufs=8) as gpool,
            tc.tile_pool(name="hrpool", bufs=16) as hrpool,
            tc.tile_pool(name="hgpool", bufs=16) as hgpool,
            tc.tile_pool(name="ph", bufs=3, space="PSUM") as phpool,
            tc.tile_pool(name="py", bufs=1, space="PSUM") as pypool,
        ):
            TH = TPC // 2
            xfA = singles.tile([P, DC * TH], dt.float32)
            xfB = singles.tile([P, DC * TH], dt.float32)
            xb = singles.tile([P, DC * TPC], dt.bfloat16)
            sel = singles.tile([P, DC * E], dt.float32)
            ssb = singles.tile([P, TT * E], dt.float32)
            sig = singles.tile([P, TT * E], dt.float32)
            m8 = singles.tile([P, TT * 8], dt.float32)
            gate = singles.tile([P, TT * E], dt.bfloat16)
            gTb = singles.tile([32, TPC], dt.bfloat16)
            y_sb = singles.tile([P, DC * TPC], dt.bfloat16)
            gTd = drampool.tile([E, TPC], dt.bfloat16)

            py = pypool.tile([P, DC, TPC], dt.float32)

            wgrp = {}
            ggrp = {}
            hgs = {}
            hrs = {}

            # PE warm-up: junk matmuls on uninitialized SBUF while the input
            # DMAs stream in, so the HAM clock gate is at 8/8 (2.4 GHz) when
            # the real matmul stream starts. Results are discarded.
            junk = singles.tile([P, TPC], dt.bfloat16)
            nc.gpsimd.memset(junk[:], 1.0)
            pj = phpool.tile([P, TPC], dt.float32, tag="ph", name="pjunk")
            for _ in range(24):
                nc.tensor.matmul(pj[:], junk[:, :P], junk[:], start=True, stop=True)

            def dma_group(gi):
                wt = wpool.tile([P, G * WREC], dt.bfloat16, tag="wt", name=f"wt{gi}")
                nc.sync.dma_start(wt[:], wc_d[gi])
                wgrp[gi] = wt

            def g_group(gi):
                g = gpool.tile([P, G * TPC], dt.bfloat16, tag="g", name=f"g{gi}")
                base = gTd[gi * G : (gi + 1) * G, :]
                src = bass.AP(base.tensor, base.offset, [[0, P]] + list(base.ap))
                nc.gpsimd.dma_start(g[:].rearrange("p (e t) -> p e t", e=G), src)
                ggrp[gi] = g

            def scores_section():
                for tt in range(TT):
                    psc = phpool.tile([P, E], dt.float32, tag="ph", name=f"psc{tt}")
                    xh = xfA if tt < 2 else xfB
                    tl = (tt % 2) * P
                    for dc in range(DC):
                        nc.tensor.matmul(
                            psc[:],
                            xh[:, dc * TH + tl : dc * TH + tl + P],
                            sel[:, dc * E : (dc + 1) * E],
                            start=(dc == 0),
                            stop=(dc == DC - 1),
                        )
                    sl = slice(tt * E, (tt + 1) * E)
                    nc.scalar.activation(sig[:, sl], psc[:], Act.Sigmoid)
                    nc.vector.tensor_copy(ssb[:, sl], psc[:])
                    nc.vector.max(m8[:, tt * 8 : (tt + 1) * 8], ssb[:, sl])
                    # gate = (score >= 4th max) * sigmoid(score), in bf16
                    nc.vector.scalar_tensor_tensor(
                        gate[:, sl],
                        ssb[:, sl],
                        m8[:, tt * 8 + 3 : tt * 8 + 4],
                        sig[:, sl],
                        op0=Alu.is_ge,
                        op1=Alu.mult,
                    )
                    # transpose this token-tile of the gate to [E, T]
                    for pb in range(TT):
                        nc.vector.transpose(
                            gTb[0:32, tt * P + pb * 32 : tt * P + (pb + 1) * 32],
                            gate[pb * 32 : (pb + 1) * 32, sl],
                        )
                nc.gpsimd.dma_start(gTd[:], gTb[0:32, :])

            def l1_mm(e):
                gi, ei = divmod(e, G)
                wt = wgrp[gi]
                ph = phpool.tile([P, TPC], dt.float32, tag="ph", name=f"ph{e}")
                for dc in range(DC):
                    nc.tensor.matmul(
                        ph[:],
                        wt[:, ei * WREC + dc * H : ei * WREC + (dc + 1) * H],
                        xb[:, dc * TPC : (dc + 1) * TPC],
                        start=(dc == 0),
                        stop=(dc == DC - 1),
                    )
                hr = hrpool.tile([P, TPC], dt.bfloat16, tag="hr", name=f"hr{e}")
                nc.scalar.activation(hr[:], ph[:], Act.Relu)
                hrs[e] = hr

            def l1_mul(e):
                gi, ei = divmod(e, G)
                hr = hrs.pop(e)
                hg = hgpool.tile([P, TPC], dt.bfloat16, tag="hg", name=f"hg{e}")
                nc.vector.tensor_mul(
                    hg[:], hr[:], ggrp[gi][:, ei * TPC : (ei + 1) * TPC]
                )
                hgs[e] = hg

            def l2_part(e):
                gi, ei = divmod(e, G)
                hg = hgs.pop(e)
                wt = wgrp[gi]
                base = ei * WREC + DC * H
                for dtile in range(DC):
                    nc.tensor.matmul(
                        py[:, dtile, :],
                        wt[:, base + dtile * P : base + (dtile + 1) * P],
                        hg[:],
                        start=(e == 0),
                        stop=(e == E - 1),
                        skip_group_check=True,
                    )

            LAG = 14
            # SP HWDGE enqueue in need order; every weight group precedes any
            # gate read so weight waits never include gate transfers. The
            # routing inputs go first: the gate path is the long pole.
            nc.sync.dma_start(sel[:], selT_d[:])
            src = xT_d[:].rearrange("p (c t) -> p c t", c=DC)
            nc.sync.dma_start(
                xfA[:].rearrange("p (c t) -> p c t", c=DC), src[:, :, :TH]
            )
            nc.sync.dma_start(
                xfB[:].rearrange("p (c t) -> p c t", c=DC), src[:, :, TH:]
            )
            nc.sync.dma_start(xb[:], xbT_d[:])
            for gi in range(NG):
                dma_group(gi)
            scores_section()
            for gi in range(NG):
                g_group(gi)
            for e in range(E + LAG):
                if e < E:
                    l1_mm(e)
                    l1_mul(e)
                if e >= LAG:
                    l2_part(e - LAG)

            # --- evict y and store (copies split across DVE/ACT) ---
            for dtile in range(DC):
                sl = slice(dtile * TPC, (dtile + 1) * TPC)
                if dtile % 2 == 0:
                    nc.vector.tensor_copy(y_sb[:, sl], py[:, dtile, :])
                else:
                    nc.scalar.activation(y_sb[:, sl], py[:, dtile, :], Act.Copy)
                nc.sync.dma_start(yT_d[dtile], y_sb[:, sl])

    return nc


def _get_nc():
    if "nc" not in _CACHE:
        _CACHE["nc"] = _build_nc()
    return _CACHE["nc"]


def _pack_inputs(x, expert_sel, w1, w2):
    x = np.asarray(x, dtype=np.float32)
    expert_sel = np.asarray(expert_sel, dtype=np.float32)
    w1 = np.asarray(w1, dtype=np.float32)
    w2 = np.asarray(w2, dtype=np.float32)

    # selT: [p, dc*E + e] = expert_sel[e, dc*P + p]
    selT = np.ascontiguousarray(
        expert_sel.T.reshape(DC, P, E).transpose(1, 0, 2)
    ).reshape(P, DC * E)
    # per-expert record [p, dc*H + h | DC*H + d], grouped by G experts
    w1p = (
        w1.astype(BF16).reshape(E, DC, P, H).transpose(0, 2, 1, 3).reshape(E, P, DC * H)
    )
    w2p = w2.astype(BF16)
    wc = np.concatenate([w1p, w2p], axis=2)  # [E, P, WREC]
    wc = np.ascontiguousarray(
        wc.reshape(NG, G, P, WREC).transpose(0, 2, 1, 3)
    ).reshape(NG, P, G * WREC)

    in_maps = []
    for c in range(NCORES):
        xc = x[c * TPC : (c + 1) * TPC]
        # xT: [p, dc*TPC + t] = x[t, dc*P + p]
        xT = np.ascontiguousarray(
            xc.T.reshape(DC, P, TPC).transpose(1, 0, 2)
        ).reshape(P, DC * TPC)
        in_maps.append({"xT": xT, "xbT": xT.astype(BF16), "selT": selT, "wc": wc})
    return in_maps


def _run(x, expert_sel, w1, w2, trace=False, tmpdir=None):
    _patch_compile()
    from concourse.bass_utils import run_bass_kernel_spmd

    if trace:
        _install_ntff_hook()

    nc = _get_nc()
    in_maps = _pack_inputs(x, expert_sel, w1, w2)
    res = run_bass_kernel_spmd(
        nc, in_maps, list(range(NCORES)), trace=trace, tmpdir=tmpdir
    )
    y = np.empty((N, D), dtype=np.float32)
    for c in range(NCORES):
        yT = np.asarray(res.results[c]["yT"], dtype=np.float32)
        y[c * TPC : (c + 1) * TPC] = yT.reshape(D, TPC).T
    return y, res


def _install_ntff_hook():
    """Register the NTFF profiling hook (the container's antenv stub lacks
    axon_hooks; replicate trn_boot's ctypes hook). Also stub the artifact
    upload, which needs cloud storage not present here."""
    if _CACHE.get("ntff"):
        return
    import sys, types, ctypes, contextlib
    import antenv  # noqa: F401
    import concourse.bass_utils as bass_utils

    bass_utils.upload_artifacts = lambda d: f"file://{d}"

    mod = types.ModuleType("antenv.axon_hooks")
    store = [None]
    mod.set_axon_ntff_profile_hook = lambda h: store.__setitem__(0, h)
    mod.get_axon_ntff_profile_hook = lambda: store[0]
    sys.modules["antenv.axon_hooks"] = mod

    lib = ctypes.CDLL("/opt/axon/libaxon_pjrt.so")
    if hasattr(lib, "axon_start_nrt_profile"):
        lib.axon_start_nrt_profile.argtypes = [
            ctypes.POINTER(ctypes.c_int64),
            ctypes.c_size_t,
        ]
        lib.axon_start_nrt_profile.restype = ctypes.c_int64
        lib.axon_stop_nrt_profile.argtypes = [ctypes.c_char_p]
        lib.axon_stop_nrt_profile.restype = ctypes.c_int64

        @contextlib.contextmanager
        def _hook(output_dir, device_ids):
            import jax

            jax.devices()
            if device_ids:
                ids = (ctypes.c_int64 * len(device_ids))(*device_ids)
                rc = lib.axon_start_nrt_profile(ids, len(device_ids))
            else:
                rc = lib.axon_start_nrt_profile(None, 0)
            if rc != 0:
                raise RuntimeError(f"axon_start_nrt_profile rc={rc}")
            try:
                yield
            finally:
                n = lib.axon_stop_nrt_profile(str(output_dir).encode())
                if n <= 0:
                    print(f"ntff profile wrote {n} files", flush=True)

        mod.set_axon_ntff_profile_hook(_hook)
    _CACHE["ntff"] = True


def kernel(x, expert_sel, w1, w2):
    y, _ = _run(x, expert_sel, w1, w2, trace=False)
    return y



# revision 5
# speedup vs baseline: 1.4828x; 1.0571x over previous
"""MoE (top-4 of 32 experts) Trainium2 kernel, data-parallel over 8 NeuronCores.

Problem shapes: x[4096,512] f32, expert_sel[32,512] f32, w1[32,512,128] f32,
w2[32,128,512] f32 -> y[4096,512] f32.

Strategy: shard the 4096 tokens 512-per-core (no collectives). Each core:
  - scores = x @ expert_sel.T in fp32 on PE (routing must match the
    reference's fp32 ranking exactly, so this stays fp32)
  - sigmoid on ACT; top-4 threshold per token via the DVE Max8 instruction;
    gate = sigmoid(score) * (score >= 4th_max), in bf16
  - gate rows are transposed to [E, T] with DVE 32x32 stream transposes,
    bounced through DRAM, and DMA-broadcast-read back as [128, T] per group
    of experts (SBUF APs cannot broadcast the partition dim; DRAM APs can)
  - dense expert MLPs in bf16: h = relu(x @ w1[e]) on ACT; hg = h * gate_e
    on DVE; y += hg @ w2[e] accumulated for all 32 experts in 4 persistent
    PSUM banks. L1 of the first experts runs before the routing chain (the
    gate-muls have LAG experts of slack).
  - all heavy DMAs are host-packed contiguous 2D transfers, enqueued on the
    SP HWDGE in need order (cumulative queue-sem thresholds then only cover
    transfers a consumer genuinely needs).
  - y written back as [D, T] tiles; host transposes and concatenates.
"""

import os
import numpy as np
import ml_dtypes

N, D, E, H, K = 4096, 512, 32, 128, 4
NCORES = 8
TPC = N // NCORES  # tokens per core = 512
P = 128
DC = D // P        # 4 contraction chunks of d
TT = TPC // P      # 4 token tiles of 128
G = 4              # experts per DMA group
NG = E // G
WREC = DC * H + D  # one expert's w1+w2 record (1024 cols)
BF16 = ml_dtypes.bfloat16

_CACHE = {}


def _split_waits_json(bir_bytes, max_waits=1):
    """The walrus build in this container encodes at most one sync-wait per
    instruction; Tile emits several on some. Split excess waits onto
    preceding same-engine NoOps (identical semantics: program order on the
    engine)."""
    import orjson

    bir = orjson.loads(bir_bytes)
    nid = [0]

    def fix_block(instructions):
        out = []
        for ins in instructions:
            si = ins.get("sync_info")
            waits = (si or {}).get("on_wait") or []
            if len(waits) > max_waits:
                chunks = [
                    waits[i : i + max_waits] for i in range(0, len(waits), max_waits)
                ]
                for ch in chunks[:-1]:
                    nid[0] += 1
                    out.append(
                        {
                            "opcode": "NoOp",
                            "engine": ins["engine"],
                            "ins": [],
                            "outs": [],
                            "name": f"I-ws-{nid[0]}",
                            "debug": ins.get("debug", 0),
                            "sync_info": {"on_wait": ch, "on_update": []},
                        }
                    )
                si["on_wait"] = chunks[-1]
            out.append(ins)
        return out

    def walk(o):
        if isinstance(o, dict):
            for k, v in o.items():
                if k == "instructions" and isinstance(v, list):
                    o[k] = fix_block(v)
                else:
                    walk(v)
        elif isinstance(o, list):
            for v in o:
                walk(v)

    walk(bir)
    return orjson.dumps(bir)


def _patch_compile():
    if _CACHE.get("patched"):
        return
    import concourse.bass2jax as bass2jax
    from concourse.bass_utils import compile_bir_kernel as _orig

    def patched(bir_json, tmpdir, neff_name="file.neff"):
        return _orig(_split_waits_json(bir_json), tmpdir, neff_name=neff_name)

    bass2jax.compile_bir_kernel = patched
    _CACHE["patched"] = True


def _patch_tile_tail():
    # Tile's kernel epilogue is drain + barrier + sem-clears + barrier. The
    # second barrier only orders the clears vs engine program-end; NRT
    # already requires every engine's program to finish before the NEFF can
    # run again, so it is dead time (~2-3us). Drop it.
    if _CACHE.get("tail_patched"):
        return
    from concourse.tile import TileContext
    from concourse.vector_clock import ScopedClock

    def _dab(self, tick_clock, wait_clock):
        drain_inst = self.nc.sync.drain()
        wait_clock.add_sem_waits(
            drain_inst.ins, ScopedClock({None: tick_clock.global_clock})
        )
        self.nc.all_engine_barrier()
        popped = self.nc._tile_sem_poison_stack.pop()
        assert popped is self._sem_poison
        self.nc.clear_and_free_semaphores(list(self.sems.allocated().values()))

    TileContext._drain_and_barrier = _dab
    _CACHE["tail_patched"] = True


def _build_nc():
    import concourse.bass as bass
    import concourse.mybir as mybir
    from concourse.tile import TileContext

    _patch_tile_tail()

    dt = mybir.dt
    Alu = mybir.AluOpType
    Act = mybir.ActivationFunctionType

    nc = bass.Bass()

    xT_d = nc.dram_tensor("xT", [P, DC * TPC], dt.float32, kind="ExternalInput")
    xbT_d = nc.dram_tensor("xbT", [P, DC * TPC], dt.bfloat16, kind="ExternalInput")
    selT_d = nc.dram_tensor("selT", [P, DC * E], dt.float32, kind="ExternalInput")
    wc_d = nc.dram_tensor("wc", [NG, P, G * WREC], dt.bfloat16, kind="ExternalInput")
    yT_d = nc.dram_tensor("yT", [DC, P, TPC], dt.bfloat16, kind="ExternalOutput")

    with TileContext(nc) as tc:
        with (
            tc.tile_pool(name="singles", bufs=1) as singles,
            tc.tile_pool(name="dram", bufs=1, space="DRAM") as drampool,
            tc.tile_pool(name="wpool", bufs=8) as wpool,
            tc.tile_pool(name="gpool", bufs=8) as gpool,
            tc.tile_pool(name="hrpool", bufs=16) as hrpool,
            tc.tile_pool(name="hgpool", bufs=16) as hgpool,
            tc.tile_pool(name="ph", bufs=3, space="PSUM") as phpool,
            tc.tile_pool(name="py", bufs=1, space="PSUM") as pypool,
        ):
            TH = TPC // 2
            xfA = singles.tile([P, DC * TH], dt.float32)
            xfB = singles.tile([P, DC * TH], dt.float32)
            xb = singles.tile([P, DC * TPC], dt.bfloat16)
            sel = singles.tile([P, DC * E], dt.float32)
            ssb = singles.tile([P, TT * E], dt.float32)
            sig = singles.tile([P, TT * E], dt.float32)
            m8 = singles.tile([P, TT * 8], dt.float32)
            gate = singles.tile([P, TT * E], dt.bfloat16)
            gTb = singles.tile([32, TPC], dt.bfloat16)
            y_sb = singles.tile([P, DC * TPC], dt.bfloat16)
            gTd = drampool.tile([E, TPC], dt.bfloat16)

            py = pypool.tile([P, DC, TPC], dt.float32)

            wgrp = {}
            ggrp = {}
            hgs = {}
            hrs = {}

            # PE warm-up: junk matmuls on uninitialized SBUF while the input
            # DMAs stream in, so the HAM clock gate is at 8/8 (2.4 GHz) when
            # the real matmul stream starts. Results are discarded.
            junk = singles.tile([P, TPC], dt.bfloat16)
            nc.gpsimd.memset(junk[:], 1.0)
            pj = phpool.tile([P, TPC], dt.float32, tag="ph", name="pjunk")
            for _ in range(24):
                nc.tensor.matmul(pj[:], junk[:, :P], junk[:], start=True, stop=True)

            def dma_group(gi):
                wt = wpool.tile([P, G * WREC], dt.bfloat16, tag="wt", name=f"wt{gi}")
                nc.sync.dma_start(wt[:], wc_d[gi])
                wgrp[gi] = wt

            def g_group(gi):
                g = gpool.tile([P, G * TPC], dt.bfloat16, tag="g", name=f"g{gi}")
                base = gTd[gi * G : (gi + 1) * G, :]
                src = bass.AP(base.tensor, base.offset, [[0, P]] + list(base.ap))
                nc.sync.dma_start(g[:].rearrange("p (e t) -> p e t", e=G), src)
                ggrp[gi] = g

            def scores_section():
                for tt in range(TT):
                    psc = phpool.tile([P, E], dt.float32, tag="ph", name=f"psc{tt}")
                    xh = xfA if tt < 2 else xfB
                    tl = (tt % 2) * P
                    for dc in range(DC):
                        nc.tensor.matmul(
                            psc[:],
                            xh[:, dc * TH + tl : dc * TH + tl + P],
                            sel[:, dc * E : (dc + 1) * E],
                            start=(dc == 0),
                            stop=(dc == DC - 1),
                        )
                    sl = slice(tt * E, (tt + 1) * E)
                    nc.scalar.activation(sig[:, sl], psc[:], Act.Sigmoid)
                    nc.vector.tensor_copy(ssb[:, sl], psc[:])
                    nc.vector.max(m8[:, tt * 8 : (tt + 1) * 8], ssb[:, sl])
                    # gate = (score >= 4th max) * sigmoid(score), in bf16
                    nc.vector.scalar_tensor_tensor(
                        gate[:, sl],
                        ssb[:, sl],
                        m8[:, tt * 8 + 3 : tt * 8 + 4],
                        sig[:, sl],
                        op0=Alu.is_ge,
                        op1=Alu.mult,
                    )
                    # transpose this token-tile of the gate to [E, T]
                    for pb in range(TT):
                        nc.vector.transpose(
                            gTb[0:32, tt * P + pb * 32 : tt * P + (pb + 1) * 32],
                            gate[pb * 32 : (pb + 1) * 32, sl],
                        )
                nc.gpsimd.dma_start(gTd[:], gTb[0:32, :])

            def l1_mm(e):
                gi, ei = divmod(e, G)
                wt = wgrp[gi]
                ph = phpool.tile([P, TPC], dt.float32, tag="ph", name=f"ph{e}")
                for dc in range(DC):
                    nc.tensor.matmul(
                        ph[:],
                        wt[:, ei * WREC + dc * H : ei * WREC + (dc + 1) * H],
                        xb[:, dc * TPC : (dc + 1) * TPC],
                        start=(dc == 0),
                        stop=(dc == DC - 1),
                    )
                hr = hrpool.tile([P, TPC], dt.bfloat16, tag="hr", name=f"hr{e}")
                nc.scalar.activation(hr[:], ph[:], Act.Relu)
                hrs[e] = hr

            def l1_mul(e):
                gi, ei = divmod(e, G)
                hr = hrs.pop(e)
                hg = hgpool.tile([P, TPC], dt.bfloat16, tag="hg", name=f"hg{e}")
                nc.vector.tensor_mul(
                    hg[:], hr[:], ggrp[gi][:, ei * TPC : (ei + 1) * TPC]
                )
                hgs[e] = hg

            def l2_part(e):
                gi, ei = divmod(e, G)
                hg = hgs.pop(e)
                wt = wgrp[gi]
                base = ei * WREC + DC * H
                for dtile in range(DC):
                    nc.tensor.matmul(
                        py[:, dtile, :],
                        wt[:, base + dtile * P : base + (dtile + 1) * P],
                        hg[:],
                        start=(e == 0),
                        stop=(e == E - 1),
                        skip_group_check=True,
                    )

            LAG = 14
            # SP HWDGE enqueue in need order; every weight group precedes any
            # gate read so weight waits never include gate transfers. The
            # routing inputs go first: the gate path is the long pole.
            nc.sync.dma_start(sel[:], selT_d[:])
            src = xT_d[:].rearrange("p (c t) -> p c t", c=DC)
            nc.sync.dma_start(
                xfA[:].rearrange("p (c t) -> p c t", c=DC), src[:, :, :TH]
            )
            nc.sync.dma_start(
                xfB[:].rearrange("p (c t) -> p c t", c=DC), src[:, :, TH:]
            )
            nc.sync.dma_start(xb[:], xbT_d[:])
            for gi in range(NG):
                dma_group(gi)
            scores_section()
            for gi in range(NG):
                g_group(gi)
            for e in range(E + LAG):
                if e < E:
                    l1_mm(e)
                    l1_mul(e)
                if e >= LAG:
                    l2_part(e - LAG)

            # --- evict y and store (copies split across DVE/ACT) ---
            for dtile in range(DC):
                sl = slice(dtile * TPC, (dtile + 1) * TPC)
                if dtile % 2 == 0:
                    nc.vector.tensor_copy(y_sb[:, sl], py[:, dtile, :])
                else:
                    nc.scalar.activation(y_sb[:, sl], py[:, dtile, :], Act.Copy)
                nc.sync.dma_start(yT_d[dtile], y_sb[:, sl])

    return nc


def _get_nc():
    if "nc" not in _CACHE:
        _CACHE["nc"] = _build_nc()
    return _CACHE["nc"]


def _pack_inputs(x, expert_sel, w1, w2):
    x = np.asarray(x, dtype=np.float32)
    expert_sel = np.asarray(expert_sel, dtype=np.float32)
    w1 = np.asarray(w1, dtype=np.float32)
    w2 = np.asarray(w2, dtype=np.float32)

    # selT: [p, dc*E + e] = expert_sel[e, dc*P + p]
    selT = np.ascontiguousarray(
        expert_sel.T.reshape(DC, P, E).transpose(1, 0, 2)
    ).reshape(P, DC * E)
    # per-expert record [p, dc*H + h | DC*H + d], grouped by G experts
    w1p = (
        w1.astype(BF16).reshape(E, DC, P, H).transpose(0, 2, 1, 3).reshape(E, P, DC * H)
    )
    w2p = w2.astype(BF16)
    wc = np.concatenate([w1p, w2p], axis=2)  # [E, P, WREC]
    wc = np.ascontiguousarray(
        wc.reshape(NG, G, P, WREC).transpose(0, 2, 1, 3)
    ).reshape(NG, P, G * WREC)

    in_maps = []
    for c in range(NCORES):
        xc = x[c * TPC : (c + 1) * TPC]
        # xT: [p, dc*TPC + t] = x[t, dc*P + p]
        xT = np.ascontiguousarray(
            xc.T.reshape(DC, P, TPC).transpose(1, 0, 2)
        ).reshape(P, DC * TPC)
        in_maps.append({"xT": xT, "xbT": xT.astype(BF16), "selT": selT, "wc": wc})
    return in_maps


def _run(x, expert_sel, w1, w2, trace=False, tmpdir=None):
    _patch_compile()
    from concourse.bass_utils import run_bass_kernel_spmd

    if trace:
        _install_ntff_hook()

    nc = _get_nc()
    in_maps = _pack_inputs(x, expert_sel, w1, w2)
    res = run_bass_kernel_spmd(
        nc, in_maps, list(range(NCORES)), trace=trace, tmpdir=tmpdir
    )
    y = np.empty((N, D), dtype=np.float32)
    for c in range(NCORES):
        yT = np.asarray(res.results[c]["yT"], dtype=np.float32)
        y[c * TPC : (c + 1) * TPC] = yT.reshape(D, TPC).T
    return y, res


def _install_ntff_hook():
    """Register the NTFF profiling hook (the container's antenv stub lacks
    axon_hooks; replicate trn_boot's ctypes hook). Also stub the artifact
    upload, which needs cloud storage not present here."""
    if _CACHE.get("ntff"):
        return
    import sys, types, ctypes, contextlib
    import antenv  # noqa: F401
    import concourse.bass_utils as bass_utils

    bass_utils.upload_artifacts = lambda d: f"file://{d}"

    mod = types.ModuleType("antenv.axon_hooks")
    store = [None]
    mod.set_axon_ntff_profile_hook = lambda h: store.__setitem__(0, h)
    mod.get_axon_ntff_profile_hook = lambda: store[0]
    sys.modules["antenv.axon_hooks"] = mod

    lib = ctypes.CDLL("/opt/axon/libaxon_pjrt.so")
    if hasattr(lib, "axon_start_nrt_profile"):
        lib.axon_start_nrt_profile.argtypes = [
            ctypes.POINTER(ctypes.c_int64),
            ctypes.c_size_t,
        ]
        lib.axon_start_nrt_profile.restype = ctypes.c_int64
        lib.axon_stop_nrt_profile.argtypes = [ctypes.c_char_p]
        lib.axon_stop_nrt_profile.restype = ctypes.c_int64

        @contextlib.contextmanager
        def _hook(output_dir, device_ids):
            import jax

            jax.devices()
            if device_ids:
                ids = (ctypes.c_int64 * len(device_ids))(*device_ids)
                rc = lib.axon_start_nrt_profile(ids, len(device_ids))
            else:
                rc = lib.axon_start_nrt_profile(None, 0)
            if rc != 0:
                raise RuntimeError(f"axon_start_nrt_profile rc={rc}")
            try:
                yield
            finally:
                n = lib.axon_stop_nrt_profile(str(output_dir).encode())
                if n <= 0:
                    print(f"ntff profile wrote {n} files", flush=True)

        mod.set_axon_ntff_profile_hook(_hook)
    _CACHE["ntff"] = True


def kernel(x, expert_sel, w1, w2):
    y, _ = _run(x, expert_sel, w1, w2, trace=False)
    return y



# revision 6
# speedup vs baseline: 1.5058x; 1.0155x over previous
"""MoE (top-4 of 32 experts) Trainium2 kernel, data-parallel over 8 NeuronCores.

Problem shapes: x[4096,512] f32, expert_sel[32,512] f32, w1[32,512,128] f32,
w2[32,128,512] f32 -> y[4096,512] f32.

Strategy: shard the 4096 tokens 512-per-core (no collectives). Each core:
  - scores = x @ expert_sel.T in fp32 on PE (routing must match the
    reference's fp32 ranking exactly, so this stays fp32)
  - sigmoid on ACT; top-4 threshold per token via the DVE Max8 instruction;
    gate = sigmoid(score) * (score >= 4th_max), in bf16
  - gate rows are transposed to [E, T] with DVE 32x32 stream transposes,
    bounced through DRAM, and DMA-broadcast-read back as [128, T] per group
    of experts (SBUF APs cannot broadcast the partition dim; DRAM APs can)
  - dense expert MLPs in bf16: h = relu(x @ w1[e]) on ACT; hg = h * gate_e
    on DVE; y += hg @ w2[e] accumulated for all 32 experts in 4 persistent
    PSUM banks. L1 of the first experts runs before the routing chain (the
    gate-muls have LAG experts of slack).
  - all heavy DMAs are host-packed contiguous 2D transfers, enqueued on the
    SP HWDGE in need order (cumulative queue-sem thresholds then only cover
    transfers a consumer genuinely needs).
  - y written back as [D, T] tiles; host transposes and concatenates.
"""

import os
import numpy as np
import ml_dtypes

N, D, E, H, K = 4096, 512, 32, 128, 4
NCORES = 8
TPC = N // NCORES  # tokens per core = 512
P = 128
DC = D // P        # 4 contraction chunks of d
TT = TPC // P      # 4 token tiles of 128
G = 4              # experts per DMA group
NG = E // G
WREC = DC * H + D  # one expert's w1+w2 record (1024 cols)
BF16 = ml_dtypes.bfloat16

_CACHE = {}


def _split_waits_json(bir_bytes, max_waits=1):
    """The walrus build in this container encodes at most one sync-wait per
    instruction; Tile emits several on some. Split excess waits onto
    preceding same-engine NoOps (identical semantics: program order on the
    engine)."""
    import orjson

    bir = orjson.loads(bir_bytes)
    nid = [0]

    def fix_block(instructions):
        out = []
        for ins in instructions:
            si = ins.get("sync_info")
            waits = (si or {}).get("on_wait") or []
            if len(waits) > max_waits:
                chunks = [
                    waits[i : i + max_waits] for i in range(0, len(waits), max_waits)
                ]
                for ch in chunks[:-1]:
                    nid[0] += 1
                    out.append(
                        {
                            "opcode": "NoOp",
                            "engine": ins["engine"],
                            "ins": [],
                            "outs": [],
                            "name": f"I-ws-{nid[0]}",
                            "debug": ins.get("debug", 0),
                            "sync_info": {"on_wait": ch, "on_update": []},
                        }
                    )
                si["on_wait"] = chunks[-1]
            out.append(ins)
        return out

    def walk(o):
        if isinstance(o, dict):
            for k, v in o.items():
                if k == "instructions" and isinstance(v, list):
                    o[k] = fix_block(v)
                else:
                    walk(v)
        elif isinstance(o, list):
            for v in o:
                walk(v)

    walk(bir)
    return orjson.dumps(bir)


def _patch_compile():
    if _CACHE.get("patched"):
        return
    import concourse.bass2jax as bass2jax
    from concourse.bass_utils import compile_bir_kernel as _orig

    def patched(bir_json, tmpdir, neff_name="file.neff"):
        return _orig(_split_waits_json(bir_json), tmpdir, neff_name=neff_name)

    bass2jax.compile_bir_kernel = patched
    _CACHE["patched"] = True


def _patch_tile_tail():
    # Tile's kernel epilogue is drain + barrier + sem-clears + barrier. The
    # second barrier only orders the clears vs engine program-end; NRT
    # already requires every engine's program to finish before the NEFF can
    # run again, so it is dead time (~2-3us). Drop it.
    if _CACHE.get("tail_patched"):
        return
    from concourse.tile import TileContext
    from concourse.vector_clock import ScopedClock

    def _dab(self, tick_clock, wait_clock):
        drain_inst = self.nc.sync.drain()
        wait_clock.add_sem_waits(
            drain_inst.ins, ScopedClock({None: tick_clock.global_clock})
        )
        self.nc.all_engine_barrier()
        popped = self.nc._tile_sem_poison_stack.pop()
        assert popped is self._sem_poison
        self.nc.clear_and_free_semaphores(list(self.sems.allocated().values()))

    TileContext._drain_and_barrier = _dab
    _CACHE["tail_patched"] = True


def _build_nc():
    import concourse.bass as bass
    import concourse.mybir as mybir
    from concourse.tile import TileContext

    _patch_tile_tail()

    dt = mybir.dt
    Alu = mybir.AluOpType
    Act = mybir.ActivationFunctionType

    nc = bass.Bass()

    xT_d = nc.dram_tensor("xT", [P, DC * TPC], dt.float32, kind="ExternalInput")
    xbT_d = nc.dram_tensor("xbT", [P, DC * TPC], dt.bfloat16, kind="ExternalInput")
    selT_d = nc.dram_tensor("selT", [P, DC * E], dt.float32, kind="ExternalInput")
    wc_d = nc.dram_tensor("wc", [NG, P, G * WREC], dt.bfloat16, kind="ExternalInput")
    yT_d = nc.dram_tensor("yT", [DC, P, TPC], dt.bfloat16, kind="ExternalOutput")

    with TileContext(nc) as tc:
        with (
            tc.tile_pool(name="singles", bufs=1) as singles,
            tc.tile_pool(name="dram", bufs=1, space="DRAM") as drampool,
            tc.tile_pool(name="wpool", bufs=8) as wpool,
            tc.tile_pool(name="gpool", bufs=8) as gpool,
            tc.tile_pool(name="hrpool", bufs=16) as hrpool,
            tc.tile_pool(name="hgpool", bufs=16) as hgpool,
            tc.tile_pool(name="ph", bufs=3, space="PSUM") as phpool,
            tc.tile_pool(name="py", bufs=1, space="PSUM") as pypool,
        ):
            TH = TPC // 2
            xfA = singles.tile([P, DC * TH], dt.float32)
            xfB = singles.tile([P, DC * TH], dt.float32)
            xb = singles.tile([P, DC * TPC], dt.bfloat16)
            sel = singles.tile([P, DC * E], dt.float32)
            ssb = singles.tile([P, TT * E], dt.float32)
            sig = singles.tile([P, TT * E], dt.float32)
            m8 = singles.tile([P, TT * 8], dt.float32)
            gate = singles.tile([P, TT * E], dt.bfloat16)
            gTb = singles.tile([32, TPC], dt.bfloat16)
            y_sb = singles.tile([P, DC * TPC], dt.bfloat16)
            gTd = drampool.tile([E, TPC], dt.bfloat16)

            py = pypool.tile([P, DC, TPC], dt.float32)

            wgrp = {}
            ggrp = {}
            hgs = {}
            hrs = {}

            # PE warm-up: junk matmuls on uninitialized SBUF while the input
            # DMAs stream in, so the HAM clock gate is at 8/8 (2.4 GHz) when
            # the real matmul stream starts. Results are discarded.
            junk = singles.tile([P, TPC], dt.bfloat16)
            nc.gpsimd.memset(junk[:], 1.0)
            pj = phpool.tile([P, TPC], dt.float32, tag="ph", name="pjunk")
            for _ in range(16):
                nc.tensor.matmul(pj[:], junk[:, :P], junk[:], start=True, stop=True)

            def dma_group(gi):
                wt = wpool.tile([P, G * WREC], dt.bfloat16, tag="wt", name=f"wt{gi}")
                nc.sync.dma_start(wt[:], wc_d[gi])
                wgrp[gi] = wt

            def g_group(gi):
                g = gpool.tile([P, G * TPC], dt.bfloat16, tag="g", name=f"g{gi}")
                base = gTd[gi * G : (gi + 1) * G, :]
                src = bass.AP(base.tensor, base.offset, [[0, P]] + list(base.ap))
                nc.sync.dma_start(g[:].rearrange("p (e t) -> p e t", e=G), src)
                ggrp[gi] = g

            def scores_section():
                for tt in range(TT):
                    psc = phpool.tile([P, E], dt.float32, tag="ph", name=f"psc{tt}")
                    xh = xfA if tt < 2 else xfB
                    tl = (tt % 2) * P
                    for dc in range(DC):
                        nc.tensor.matmul(
                            psc[:],
                            xh[:, dc * TH + tl : dc * TH + tl + P],
                            sel[:, dc * E : (dc + 1) * E],
                            start=(dc == 0),
                            stop=(dc == DC - 1),
                        )
                    sl = slice(tt * E, (tt + 1) * E)
                    nc.scalar.activation(sig[:, sl], psc[:], Act.Sigmoid)
                    nc.vector.tensor_copy(ssb[:, sl], psc[:])
                    nc.vector.max(m8[:, tt * 8 : (tt + 1) * 8], ssb[:, sl])
                    # gate = (score >= 4th max) * sigmoid(score), in bf16
                    nc.vector.scalar_tensor_tensor(
                        gate[:, sl],
                        ssb[:, sl],
                        m8[:, tt * 8 + 3 : tt * 8 + 4],
                        sig[:, sl],
                        op0=Alu.is_ge,
                        op1=Alu.mult,
                    )
                    # transpose this token-tile of the gate to [E, T]
                    for pb in range(TT):
                        nc.vector.transpose(
                            gTb[0:32, tt * P + pb * 32 : tt * P + (pb + 1) * 32],
                            gate[pb * 32 : (pb + 1) * 32, sl],
                        )
                nc.gpsimd.dma_start(gTd[:], gTb[0:32, :])

            def l1_mm(e):
                gi, ei = divmod(e, G)
                wt = wgrp[gi]
                ph = phpool.tile([P, TPC], dt.float32, tag="ph", name=f"ph{e}")
                for dc in range(DC):
                    nc.tensor.matmul(
                        ph[:],
                        wt[:, ei * WREC + dc * H : ei * WREC + (dc + 1) * H],
                        xb[:, dc * TPC : (dc + 1) * TPC],
                        start=(dc == 0),
                        stop=(dc == DC - 1),
                    )
                hr = hrpool.tile([P, TPC], dt.bfloat16, tag="hr", name=f"hr{e}")
                nc.scalar.activation(hr[:], ph[:], Act.Relu)
                hrs[e] = hr

            def l1_mul(e):
                gi, ei = divmod(e, G)
                hr = hrs.pop(e)
                hg = hgpool.tile([P, TPC], dt.bfloat16, tag="hg", name=f"hg{e}")
                nc.vector.tensor_mul(
                    hg[:], hr[:], ggrp[gi][:, ei * TPC : (ei + 1) * TPC]
                )
                hgs[e] = hg

            def l2_part(e):
                gi, ei = divmod(e, G)
                hg = hgs.pop(e)
                wt = wgrp[gi]
                base = ei * WREC + DC * H
                for dtile in range(DC):
                    nc.tensor.matmul(
                        py[:, dtile, :],
                        wt[:, base + dtile * P : base + (dtile + 1) * P],
                        hg[:],
                        start=(e == 0),
                        stop=(e == E - 1),
                        skip_group_check=True,
                    )

            LAG = 14
            # SP HWDGE enqueue in need order; every weight group precedes any
            # gate read so weight waits never include gate transfers. The
            # routing inputs go first: the gate path is the long pole.
            nc.sync.dma_start(sel[:], selT_d[:])
            src = xT_d[:].rearrange("p (c t) -> p c t", c=DC)
            nc.sync.dma_start(
                xfA[:].rearrange("p (c t) -> p c t", c=DC), src[:, :, :TH]
            )
            nc.sync.dma_start(
                xfB[:].rearrange("p (c t) -> p c t", c=DC), src[:, :, TH:]
            )
            nc.sync.dma_start(xb[:], xbT_d[:])
            for gi in range(5):
                dma_group(gi)
            scores_section()
            g_group(0)
            g_group(1)
            dma_group(5)
            g_group(2)
            dma_group(6)
            g_group(3)
            dma_group(7)
            for gi in range(4, NG):
                g_group(gi)
            for e in range(E + LAG):
                if e < E:
                    l1_mm(e)
                    l1_mul(e)
                if e >= LAG:
                    l2_part(e - LAG)

            # --- evict y and store (copies split across DVE/ACT) ---
            for dtile in range(DC):
                sl = slice(dtile * TPC, (dtile + 1) * TPC)
                if dtile % 2 == 0:
                    nc.vector.tensor_copy(y_sb[:, sl], py[:, dtile, :])
                else:
                    nc.scalar.activation(y_sb[:, sl], py[:, dtile, :], Act.Copy)
                nc.sync.dma_start(yT_d[dtile], y_sb[:, sl])

    return nc


def _get_nc():
    if "nc" not in _CACHE:
        _CACHE["nc"] = _build_nc()
    return _CACHE["nc"]


def _pack_inputs(x, expert_sel, w1, w2):
    x = np.asarray(x, dtype=np.float32)
    expert_sel = np.asarray(expert_sel, dtype=np.float32)
    w1 = np.asarray(w1, dtype=np.float32)
    w2 = np.asarray(w2, dtype=np.float32)

    # selT: [p, dc*E + e] = expert_sel[e, dc*P + p]
    selT = np.ascontiguousarray(
        expert_sel.T.reshape(DC, P, E).transpose(1, 0, 2)
    ).reshape(P, DC * E)
    # per-expert record [p, dc*H + h | DC*H + d], grouped by G experts
    w1p = (
        w1.astype(BF16).reshape(E, DC, P, H).transpose(0, 2, 1, 3).reshape(E, P, DC * H)
    )
    w2p = w2.astype(BF16)
    wc = np.concatenate([w1p, w2p], axis=2)  # [E, P, WREC]
    wc = np.ascontiguousarray(
        wc.reshape(NG, G, P, WREC).transpose(0, 2, 1, 3)
    ).reshape(NG, P, G * WREC)

    in_maps = []
    for c in range(NCORES):
        xc = x[c * TPC : (c + 1) * TPC]
        # xT: [p, dc*TPC + t] = x[t, dc*P + p]
        xT = np.ascontiguousarray(
            xc.T.reshape(DC, P, TPC).transpose(1, 0, 2)
        ).reshape(P, DC * TPC)
        in_maps.append({"xT": xT, "xbT": xT.astype(BF16), "selT": selT, "wc": wc})
    return in_maps


def _run(x, expert_sel, w1, w2, trace=False, tmpdir=None):
    _patch_compile()
    from concourse.bass_utils import run_bass_kernel_spmd

    if trace:
        _install_ntff_hook()

    nc = _get_nc()
    in_maps = _pack_inputs(x, expert_sel, w1, w2)
    res = run_bass_kernel_spmd(
        nc, in_maps, list(range(NCORES)), trace=trace, tmpdir=tmpdir
    )
    y = np.empty((N, D), dtype=np.float32)
    for c in range(NCORES):
        yT = np.asarray(res.results[c]["yT"], dtype=np.float32)
        y[c * TPC : (c + 1) * TPC] = yT.reshape(D, TPC).T
    return y, res


def _install_ntff_hook():
    """Register the NTFF profiling hook (the container's antenv stub lacks
    axon_hooks; replicate trn_boot's ctypes hook). Also stub the artifact
    upload, which needs cloud storage not present here."""
    if _CACHE.get("ntff"):
        return
    import sys, types, ctypes, contextlib
    import antenv  # noqa: F401
    import concourse.bass_utils as bass_utils

    bass_utils.upload_artifacts = lambda d: f"file://{d}"

    mod = types.ModuleType("antenv.axon_hooks")
    store = [None]
    mod.set_axon_ntff_profile_hook = lambda h: store.__setitem__(0, h)
    mod.get_axon_ntff_profile_hook = lambda: store[0]
    sys.modules["antenv.axon_hooks"] = mod

    lib = ctypes.CDLL("/opt/axon/libaxon_pjrt.so")
    if hasattr(lib, "axon_start_nrt_profile"):
        lib.axon_start_nrt_profile.argtypes = [
            ctypes.POINTER(ctypes.c_int64),
            ctypes.c_size_t,
        ]
        lib.axon_start_nrt_profile.restype = ctypes.c_int64
        lib.axon_stop_nrt_profile.argtypes = [ctypes.c_char_p]
        lib.axon_stop_nrt_profile.restype = ctypes.c_int64

        @contextlib.contextmanager
        def _hook(output_dir, device_ids):
            import jax

            jax.devices()
            if device_ids:
                ids = (ctypes.c_int64 * len(device_ids))(*device_ids)
                rc = lib.axon_start_nrt_profile(ids, len(device_ids))
            else:
                rc = lib.axon_start_nrt_profile(None, 0)
            if rc != 0:
                raise RuntimeError(f"axon_start_nrt_profile rc={rc}")
            try:
                yield
            finally:
                n = lib.axon_stop_nrt_profile(str(output_dir).encode())
                if n <= 0:
                    print(f"ntff profile wrote {n} files", flush=True)

        mod.set_axon_ntff_profile_hook(_hook)
    _CACHE["ntff"] = True


def kernel(x, expert_sel, w1, w2):
    y, _ = _run(x, expert_sel, w1, w2, trace=False)
    return y



# revision 7
# speedup vs baseline: 1.5321x; 1.0175x over previous
"""MoE (top-4 of 32 experts) Trainium2 kernel, data-parallel over 8 NeuronCores.

Problem shapes: x[4096,512] f32, expert_sel[32,512] f32, w1[32,512,128] f32,
w2[32,128,512] f32 -> y[4096,512] f32.

Strategy: shard the 4096 tokens 512-per-core (no collectives). Each core:
  - scores = x @ expert_sel.T in fp32 on PE (routing must match the
    reference's fp32 ranking exactly, so this stays fp32)
  - sigmoid on ACT; top-4 threshold per token via the DVE Max8 instruction;
    gate = sigmoid(score) * (score >= 4th_max), in bf16
  - gate rows are transposed to [E, T] with DVE 32x32 stream transposes,
    bounced through DRAM, and DMA-broadcast-read back as [128, T] per group
    of experts (SBUF APs cannot broadcast the partition dim; DRAM APs can)
  - dense expert MLPs in bf16: h = relu(x @ w1[e]) on ACT; hg = h * gate_e
    on DVE; y += hg @ w2[e] accumulated for all 32 experts in 4 persistent
    PSUM banks. L1 of the first experts runs before the routing chain (the
    gate-muls have LAG experts of slack).
  - all heavy DMAs are host-packed contiguous 2D transfers, enqueued on the
    SP HWDGE in need order (cumulative queue-sem thresholds then only cover
    transfers a consumer genuinely needs).
  - y written back as [D, T] tiles; host transposes and concatenates.
"""

import os
import numpy as np
import ml_dtypes

N, D, E, H, K = 4096, 512, 32, 128, 4
NCORES = 8
TPC = N // NCORES  # tokens per core = 512
P = 128
DC = D // P        # 4 contraction chunks of d
TT = TPC // P      # 4 token tiles of 128
G = 4              # experts per DMA group
NG = E // G
WREC = DC * H + D  # one expert's w1+w2 record (1024 cols)
BF16 = ml_dtypes.bfloat16

_CACHE = {}


def _split_waits_json(bir_bytes, max_waits=1):
    """The walrus build in this container encodes at most one sync-wait per
    instruction; Tile emits several on some. Split excess waits onto
    preceding same-engine NoOps (identical semantics: program order on the
    engine)."""
    import orjson

    bir = orjson.loads(bir_bytes)
    nid = [0]

    def fix_block(instructions):
        out = []
        for ins in instructions:
            si = ins.get("sync_info")
            waits = (si or {}).get("on_wait") or []
            if len(waits) > max_waits:
                chunks = [
                    waits[i : i + max_waits] for i in range(0, len(waits), max_waits)
                ]
                for ch in chunks[:-1]:
                    nid[0] += 1
                    out.append(
                        {
                            "opcode": "NoOp",
                            "engine": ins["engine"],
                            "ins": [],
                            "outs": [],
                            "name": f"I-ws-{nid[0]}",
                            "debug": ins.get("debug", 0),
                            "sync_info": {"on_wait": ch, "on_update": []},
                        }
                    )
                si["on_wait"] = chunks[-1]
            out.append(ins)
        return out

    def walk(o):
        if isinstance(o, dict):
            for k, v in o.items():
                if k == "instructions" and isinstance(v, list):
                    o[k] = fix_block(v)
                else:
                    walk(v)
        elif isinstance(o, list):
            for v in o:
                walk(v)

    walk(bir)
    return orjson.dumps(bir)


def _patch_compile():
    if _CACHE.get("patched"):
        return
    import concourse.bass2jax as bass2jax
    from concourse.bass_utils import compile_bir_kernel as _orig

    def patched(bir_json, tmpdir, neff_name="file.neff"):
        return _orig(_split_waits_json(bir_json), tmpdir, neff_name=neff_name)

    bass2jax.compile_bir_kernel = patched
    _CACHE["patched"] = True


def _patch_tile_tail():
    # Tile's kernel epilogue is drain + barrier + sem-clears + barrier. The
    # second barrier only orders the clears vs engine program-end; NRT
    # already requires every engine's program to finish before the NEFF can
    # run again, so it is dead time (~2-3us). Drop it.
    if _CACHE.get("tail_patched"):
        return
    from concourse.tile import TileContext
    from concourse.vector_clock import ScopedClock

    def _dab(self, tick_clock, wait_clock):
        drain_inst = self.nc.sync.drain()
        wait_clock.add_sem_waits(
            drain_inst.ins, ScopedClock({None: tick_clock.global_clock})
        )
        self.nc.all_engine_barrier()
        popped = self.nc._tile_sem_poison_stack.pop()
        assert popped is self._sem_poison
        self.nc.clear_and_free_semaphores(list(self.sems.allocated().values()))

    TileContext._drain_and_barrier = _dab
    _CACHE["tail_patched"] = True


def _build_nc():
    import concourse.bass as bass
    import concourse.mybir as mybir
    from concourse.tile import TileContext

    _patch_tile_tail()

    dt = mybir.dt
    Alu = mybir.AluOpType
    Act = mybir.ActivationFunctionType

    nc = bass.Bass()

    xT_d = nc.dram_tensor("xT", [P, DC * TPC], dt.float32, kind="ExternalInput")
    xbT_d = nc.dram_tensor("xbT", [P, DC * TPC], dt.bfloat16, kind="ExternalInput")
    selT_d = nc.dram_tensor("selT", [P, DC * E], dt.float32, kind="ExternalInput")
    wc_d = nc.dram_tensor("wc", [NG, P, G * WREC], dt.bfloat16, kind="ExternalInput")
    yT_d = nc.dram_tensor("yT", [DC, P, TPC], dt.bfloat16, kind="ExternalOutput")

    with TileContext(nc) as tc:
        with (
            tc.tile_pool(name="singles", bufs=1) as singles,
            tc.tile_pool(name="dram", bufs=1, space="DRAM") as drampool,
            tc.tile_pool(name="wpool", bufs=8) as wpool,
            tc.tile_pool(name="gpool", bufs=8) as gpool,
            tc.tile_pool(name="hrpool", bufs=16) as hrpool,
            tc.tile_pool(name="hgpool", bufs=16) as hgpool,
            tc.tile_pool(name="ph", bufs=3, space="PSUM") as phpool,
            tc.tile_pool(name="py", bufs=1, space="PSUM") as pypool,
        ):
            TH = TPC // 2
            xfA = singles.tile([P, DC * TH], dt.float32)
            xfB = singles.tile([P, DC * TH], dt.float32)
            xb = singles.tile([P, DC * TPC], dt.bfloat16)
            sel = singles.tile([P, DC * E], dt.float32)
            ssb = singles.tile([P, TT * E], dt.float32)
            sig = singles.tile([P, TT * E], dt.float32)
            m8 = singles.tile([P, TT * 8], dt.float32)
            gate = singles.tile([P, TT * E], dt.bfloat16)
            gTb = singles.tile([32, TPC], dt.bfloat16)
            y_sb = singles.tile([P, DC * TPC], dt.bfloat16)
            gTd = drampool.tile([E, TPC], dt.bfloat16)

            py = pypool.tile([P, DC, TPC], dt.float32)

            wgrp = {}
            ggrp = {}
            hgs = {}
            hrs = {}

            # PE warm-up: junk matmuls on uninitialized SBUF while the input
            # DMAs stream in, so the HAM clock gate is at 8/8 (2.4 GHz) when
            # the real matmul stream starts. Results are discarded.
            junk = singles.tile([P, TPC], dt.bfloat16)
            nc.vector.memset(junk[:], 1.0)
            pj = phpool.tile([P, TPC], dt.float32, tag="ph", name="pjunk")
            for _ in range(12):
                nc.tensor.matmul(pj[:], junk[:, :P], junk[:], start=True, stop=True)

            def dma_group(gi):
                wt = wpool.tile([P, G * WREC], dt.bfloat16, tag="wt", name=f"wt{gi}")
                nc.sync.dma_start(wt[:], wc_d[gi])
                wgrp[gi] = wt

            def g_group(gi):
                g = gpool.tile([P, G * TPC], dt.bfloat16, tag="g", name=f"g{gi}")
                base = gTd[gi * G : (gi + 1) * G, :]
                src = bass.AP(base.tensor, base.offset, [[0, P]] + list(base.ap))
                nc.sync.dma_start(g[:].rearrange("p (e t) -> p e t", e=G), src)
                ggrp[gi] = g

            def scores_section():
                for tt in range(TT):
                    psc = phpool.tile([P, E], dt.float32, tag="ph", name=f"psc{tt}")
                    xh = xfA if tt < 2 else xfB
                    tl = (tt % 2) * P
                    for dc in range(DC):
                        nc.tensor.matmul(
                            psc[:],
                            xh[:, dc * TH + tl : dc * TH + tl + P],
                            sel[:, dc * E : (dc + 1) * E],
                            start=(dc == 0),
                            stop=(dc == DC - 1),
                        )
                    sl = slice(tt * E, (tt + 1) * E)
                    nc.scalar.activation(sig[:, sl], psc[:], Act.Sigmoid)
                    nc.vector.tensor_copy(ssb[:, sl], psc[:])
                    nc.vector.max(m8[:, tt * 8 : (tt + 1) * 8], ssb[:, sl])
                    # gate = (score >= 4th max) * sigmoid(score), in bf16
                    nc.vector.scalar_tensor_tensor(
                        gate[:, sl],
                        ssb[:, sl],
                        m8[:, tt * 8 + 3 : tt * 8 + 4],
                        sig[:, sl],
                        op0=Alu.is_ge,
                        op1=Alu.mult,
                    )
                    # transpose this token-tile of the gate to [E, T]
                    for pb in range(TT):
                        nc.vector.transpose(
                            gTb[0:32, tt * P + pb * 32 : tt * P + (pb + 1) * 32],
                            gate[pb * 32 : (pb + 1) * 32, sl],
                        )
                nc.gpsimd.dma_start(gTd[:], gTb[0:32, :])

            def l1_mm(e):
                gi, ei = divmod(e, G)
                wt = wgrp[gi]
                ph = phpool.tile([P, TPC], dt.float32, tag="ph", name=f"ph{e}")
                for dc in range(DC):
                    nc.tensor.matmul(
                        ph[:],
                        wt[:, ei * WREC + dc * H : ei * WREC + (dc + 1) * H],
                        xb[:, dc * TPC : (dc + 1) * TPC],
                        start=(dc == 0),
                        stop=(dc == DC - 1),
                    )
                hr = hrpool.tile([P, TPC], dt.bfloat16, tag="hr", name=f"hr{e}")
                nc.scalar.activation(hr[:], ph[:], Act.Relu)
                hrs[e] = hr

            def l1_mul(e):
                gi, ei = divmod(e, G)
                hr = hrs.pop(e)
                hg = hgpool.tile([P, TPC], dt.bfloat16, tag="hg", name=f"hg{e}")
                nc.vector.tensor_mul(
                    hg[:], hr[:], ggrp[gi][:, ei * TPC : (ei + 1) * TPC]
                )
                hgs[e] = hg

            def l2_part(e):
                gi, ei = divmod(e, G)
                hg = hgs.pop(e)
                wt = wgrp[gi]
                base = ei * WREC + DC * H
                for dtile in range(DC):
                    nc.tensor.matmul(
                        py[:, dtile, :],
                        wt[:, base + dtile * P : base + (dtile + 1) * P],
                        hg[:],
                        start=(e == 0),
                        stop=(e == E - 1),
                        skip_group_check=True,
                    )

            LAG = 14
            # SP HWDGE enqueue in need order; every weight group precedes any
            # gate read so weight waits never include gate transfers. The
            # routing inputs go first: the gate path is the long pole.
            nc.sync.dma_start(sel[:], selT_d[:])
            src = xT_d[:].rearrange("p (c t) -> p c t", c=DC)
            nc.sync.dma_start(
                xfA[:].rearrange("p (c t) -> p c t", c=DC), src[:, :, :TH]
            )
            nc.sync.dma_start(
                xfB[:].rearrange("p (c t) -> p c t", c=DC), src[:, :, TH:]
            )
            nc.sync.dma_start(xb[:], xbT_d[:])
            for gi in range(5):
                dma_group(gi)
            scores_section()
            g_group(0)
            g_group(1)
            dma_group(5)
            g_group(2)
            dma_group(6)
            g_group(3)
            dma_group(7)
            for gi in range(4, NG):
                g_group(gi)
            for e in range(E + LAG):
                if e < E:
                    l1_mm(e)
                    l1_mul(e)
                if e >= LAG:
                    l2_part(e - LAG)

            # --- evict y and store (copies split across DVE/ACT) ---
            for dtile in range(DC):
                sl = slice(dtile * TPC, (dtile + 1) * TPC)
                if dtile % 2 == 0:
                    nc.vector.tensor_copy(y_sb[:, sl], py[:, dtile, :])
                else:
                    nc.scalar.activation(y_sb[:, sl], py[:, dtile, :], Act.Copy)
                nc.sync.dma_start(yT_d[dtile], y_sb[:, sl])

    return nc


def _get_nc():
    if "nc" not in _CACHE:
        _CACHE["nc"] = _build_nc()
    return _CACHE["nc"]


def _pack_inputs(x, expert_sel, w1, w2):
    x = np.asarray(x, dtype=np.float32)
    expert_sel = np.asarray(expert_sel, dtype=np.float32)
    w1 = np.asarray(w1, dtype=np.float32)
    w2 = np.asarray(w2, dtype=np.float32)

    # selT: [p, dc*E + e] = expert_sel[e, dc*P + p]
    selT = np.ascontiguousarray(
        expert_sel.T.reshape(DC, P, E).transpose(1, 0, 2)
    ).reshape(P, DC * E)
    # per-expert record [p, dc*H + h | DC*H + d], grouped by G experts
    w1p = (
        w1.astype(BF16).reshape(E, DC, P, H).transpose(0, 2, 1, 3).reshape(E, P, DC * H)
    )
    w2p = w2.astype(BF16)
    wc = np.concatenate([w1p, w2p], axis=2)  # [E, P, WREC]
    wc = np.ascontiguousarray(
        wc.reshape(NG, G, P, WREC).transpose(0, 2, 1, 3)
    ).reshape(NG, P, G * WREC)

    in_maps = []
    for c in range(NCORES):
        xc = x[c * TPC : (c + 1) * TPC]
        # xT: [p, dc*TPC + t] = x[t, dc*P + p]
        xT = np.ascontiguousarray(
            xc.T.reshape(DC, P, TPC).transpose(1, 0, 2)
        ).reshape(P, DC * TPC)
        in_maps.append({"xT": xT, "xbT": xT.astype(BF16), "selT": selT, "wc": wc})
    return in_maps


def _run(x, expert_sel, w1, w2, trace=False, tmpdir=None):
    _patch_compile()
    from concourse.bass_utils import run_bass_kernel_spmd

    if trace:
        _install_ntff_hook()

    nc = _get_nc()
    in_maps = _pack_inputs(x, expert_sel, w1, w2)
    res = run_bass_kernel_spmd(
        nc, in_maps, list(range(NCORES)), trace=trace, tmpdir=tmpdir
    )
    y = np.empty((N, D), dtype=np.float32)
    for c in range(NCORES):
        yT = np.asarray(res.results[c]["yT"], dtype=np.float32)
        y[c * TPC : (c + 1) * TPC] = yT.reshape(D, TPC).T
    return y, res


def _install_ntff_hook():
    """Register the NTFF profiling hook (the container's antenv stub lacks
    axon_hooks; replicate trn_boot's ctypes hook). Also stub the artifact
    upload, which needs cloud storage not present here."""
    if _CACHE.get("ntff"):
        return
    import sys, types, ctypes, contextlib
    import antenv  # noqa: F401
    import concourse.bass_utils as bass_utils

    bass_utils.upload_artifacts = lambda d: f"file://{d}"

    mod = types.ModuleType("antenv.axon_hooks")
    store = [None]
    mod.set_axon_ntff_profile_hook = lambda h: store.__setitem__(0, h)
    mod.get_axon_ntff_profile_hook = lambda: store[0]
    sys.modules["antenv.axon_hooks"] = mod

    lib = ctypes.CDLL("/opt/axon/libaxon_pjrt.so")
    if hasattr(lib, "axon_start_nrt_profile"):
        lib.axon_start_nrt_profile.argtypes = [
            ctypes.POINTER(ctypes.c_int64),
            ctypes.c_size_t,
        ]
        lib.axon_start_nrt_profile.restype = ctypes.c_int64
        lib.axon_stop_nrt_profile.argtypes = [ctypes.c_char_p]
        lib.axon_stop_nrt_profile.restype = ctypes.c_int64

        @contextlib.contextmanager
        def _hook(output_dir, device_ids):
            import jax

            jax.devices()
            if device_ids:
                ids = (ctypes.c_int64 * len(device_ids))(*device_ids)
                rc = lib.axon_start_nrt_profile(ids, len(device_ids))
            else:
                rc = lib.axon_start_nrt_profile(None, 0)
            if rc != 0:
                raise RuntimeError(f"axon_start_nrt_profile rc={rc}")
            try:
                yield
            finally:
                n = lib.axon_stop_nrt_profile(str(output_dir).encode())
                if n <= 0:
                    print(f"ntff profile wrote {n} files", flush=True)

        mod.set_axon_ntff_profile_hook(_hook)
    _CACHE["ntff"] = True


def kernel(x, expert_sel, w1, w2):
    y, _ = _run(x, expert_sel, w1, w2, trace=False)
    return y

